# revision 2
# baseline (speedup 1.0000x reference)
"""Trainium2 Bass kernel for grouped-top-k MoE with shared expert (8 NeuronCores, SPMD).

Zero-collective design
----------------------
The reference's "dispatch" gathers rows of x by *expert id* (values 0..7), so the
routed path only ever reads x[0:8] and scatter-adds into output rows 0..7.  Writing
routed_out row i as g(w_i * x[t_i]; e_i) with t_i = chosen expert of assignment i and
e_i = ragged-segment expert of global row i, the whole routed computation factors
through a 64-row table:
    a[t,e] = x[t] @ w1[e],  b[t,e] = x[t] @ w3[e]            (host precompute)
    H[t,e] = sum_{i: t_i=t, e_i=e} silu(w_i*a[t,e]) * (w_i*b[t,e])
    delta[t] = sum_e H[t,e] @ w2[e];   out[t] += delta[t]  (t < 8, host combine)

No collectives at all (v1's three collectives cost 75us + a 42us rendezvous
barrier for <160KB of payload):
  - every core computes the GATE for all 4096 tokens, so global expert
    counts/offsets are available locally.  The gate matmuls run in fp8
    (DoubleRow, 2x) on a 16x-prescaled w_gate; the softmax Exp rescales.
    Routing differs from the f32 reference on ~160/4096 near-tie tokens, which
    only perturbs the 8 delta rows (measured ~0.009 total rel err vs 2e-2 gate).
  - the 64x512 a/b tables are computed on host during input packing and DMA'd.
  - each core emits its partial H [64,512] f32; the host sums the 8 partials
    and applies the tiny w2 GEMM in f32 during unshard (the v1 kernel already
    host-summed partial deltas).
  - data-parallel over tokens for the shared-expert FFN (512 tokens/core);
    per-core x is packed with the OWN 512-token block first so one NEFF serves
    all cores.

Scheduling: one statically interleaved PE stream ordered by DMA arrival -
h1 tiles (sw1t) first with gate chunks riding the fp8 x blocks, h3 tiles after
sw3t, alternating tags so PSUM rotations never stall; the gate DVE chain runs
in 4 incremental passes so counts are ready right after the last gate chunk;
phi/H then out-GEMMs form the tail.
"""

import sys

if "/opt/trn_rl_repo" not in sys.path:
    sys.path.insert(0, "/opt/trn_rl_repo")

import numpy as np
import ml_dtypes

import concourse.bass as bass
import concourse.mybir as mybir
import concourse.tile as tile
from concourse import bacc
from concourse import bass_utils

F32 = mybir.dt.float32
BF16 = mybir.dt.bfloat16
FP8 = mybir.dt.float8e4
DR = mybir.MatmulPerfMode.DoubleRow
GS = 16.0   # gate fp8 weight pre-scale (softmax Exp divides it back out)
AF = mybir.ActivationFunctionType
ALU = mybir.AluOpType
X = mybir.AxisListType.X

E = 8          # experts (== table token count == cores)
G = 4          # expert groups
D = 1024       # model dim
HID = 512      # expert hidden
SH = 1024      # shared-expert hidden
C = 8          # cores
TC = 512       # tokens per core
NTOK = 4096
NB = 32        # 128-token blocks globally
BIG = 1.0e30


def ts(i, s):
    return slice(i * s, (i + 1) * s)


def build():
    nc = bacc.Bacc("TRN2", target_bir_lowering=False, debug=False, num_devices=C)

    # ---- I/O: packed partition-major; contraction dim = k*128+p
    wg8 = nc.dram_tensor("wg8", [128, 8, 2 * E], FP8, kind="ExternalInput")
    biasd = nc.dram_tensor("biasd", [1, E], F32, kind="ExternalInput")
    ivec = nc.dram_tensor("ivec", [128, 1], F32, kind="ExternalInput")
    # all 4096 tokens in fp8 (gate only), dim-major; block 0 = own shard
    x8a = nc.dram_tensor("x8a", [128, 8, 8 * TC], FP8, kind="ExternalInput")  # block-major
    xtb = nc.dram_tensor("xtb", [128, 8, TC], BF16, kind="ExternalInput")
    sw1t = nc.dram_tensor("sw1t", [128, 8, SH], BF16, kind="ExternalInput")
    sw3t = nc.dram_tensor("sw3t", [128, 8, SH], BF16, kind="ExternalInput")
    sw2t = nc.dram_tensor("sw2t", [128, 8, D], BF16, kind="ExternalInput")
    tabs_d = nc.dram_tensor("tabs", [E * E, 2 * HID], BF16, kind="ExternalInput")
    out = nc.dram_tensor("out", [D, TC], BF16, kind="ExternalOutput")  # shared^T shard
    hout = nc.dram_tensor("hout", [E * E, HID], F32, kind="ExternalOutput")  # partial H

    # ---- compile-time constants (embedded in NEFF)
    idbf_d = nc.inline_tensor(np.eye(128, dtype=ml_dtypes.bfloat16), name="idbf")
    id8f_d = nc.inline_tensor(np.eye(E, dtype=np.float32), name="id8f")
    # negL8[k, e] = -1 if k <= e else 0;  noffs[e] = sum_k negL8[k,e]*cnt[k]
    negL8_d = nc.inline_tensor(
        np.ascontiguousarray(-np.tril(np.ones((E, E), np.float32)).T), name="negL8")
    ones8_d = nc.inline_tensor(np.ones((E, 128), np.float32), name="ones8x128")
    crow_d = nc.inline_tensor(
        np.array([[256 * Jb + k for Jb in range(4) for k in range(2)]],
                 np.float32).reshape(1, E), name="crow")

    with tile.TileContext(nc) as tc:
        with (
            tc.tile_pool(name="wp", bufs=1) as wp,       # persistent SBUF
            tc.tile_pool(name="gp", bufs=1) as gp,       # gate outputs (persist to phi)
            tc.tile_pool(name="wk", bufs=2) as wk,       # transient SBUF
            tc.tile_pool(name="psg", bufs=1, space="PSUM") as psg,   # transposes/misc
            tc.tile_pool(name="psl", bufs=1, space="PSUM") as psl,   # gate logit chunks
            tc.tile_pool(name="psh", bufs=1, space="PSUM") as psh,   # H accumulator
            tc.tile_pool(name="ps1", bufs=3, space="PSUM") as ps1,   # h1 + out tiles
            tc.tile_pool(name="ps3", bufs=2, space="PSUM") as ps3,   # h3 + phi a/b
        ):
            # ===== loads, ordered by first use (kt-halves so tiles start early)
            wg8_sb = wp.tile([128, 8, 2 * E], FP8, tag="wg8")
            nc.sync.dma_start(wg8_sb, wg8.ap())
            sw1t_sb = wp.tile([128, 8, SH], BF16, tag="sw1t")
            xtb_sb = wp.tile([128, 8, TC], BF16, tag="xtb")
            for q in range(4):
                nc.sync.dma_start(sw1t_sb[:, 2 * q:2 * q + 2, :],
                                  sw1t.ap()[:, 2 * q:2 * q + 2, :])
                nc.sync.dma_start(xtb_sb[:, 2 * q:2 * q + 2, :],
                                  xtb.ap()[:, 2 * q:2 * q + 2, :])
            x8a_sb = wp.tile([128, 8, 8 * TC], FP8, tag="x8a")  # [p, blk, kt*tok]
            nc.sync.dma_start(x8a_sb[:, 0, :], x8a.ap()[:, 0, :])
            id8f_sb = wp.tile([E, E], F32, tag="id8f")
            nc.sync.dma_start(id8f_sb, id8f_d.ap())
            bias_sb = wp.tile([128, E], F32, tag="bias")
            nc.sync.dma_start(bias_sb, biasd.ap().to_broadcast([128, E]))
            for j in range(1, 4):
                nc.sync.dma_start(x8a_sb[:, j, :], x8a.ap()[:, j, :])
            for j in range(4, 8):
                nc.sync.dma_start(x8a_sb[:, j, :], x8a.ap()[:, j, :])
            sw3t_sb = wp.tile([128, 8, SH], BF16, tag="sw3t")
            nc.sync.dma_start(sw3t_sb, sw3t.ap())
            tabs = wp.tile([E * E, 2 * HID], BF16, tag="tabs")
            nc.sync.dma_start(tabs, tabs_d.ap())
            sw2t_sb = wp.tile([128, 8, D], BF16, tag="sw2t")
            nc.sync.dma_start(sw2t_sb, sw2t.ap())
            # small late-use constants
            ivec_sb = wp.tile([128, 1], F32, tag="ivec")
            nc.sync.dma_start(ivec_sb, ivec.ap())
            idbf_sb = wp.tile([128, 128], BF16, tag="idbf")
            nc.sync.dma_start(idbf_sb, idbf_d.ap())
            negL8_sb = wp.tile([E, E], F32, tag="negL8")
            nc.sync.dma_start(negL8_sb, negL8_d.ap())
            ones8_sb = wp.tile([E, 128], F32, tag="ones8")
            nc.sync.dma_start(ones8_sb, ones8_d.ap())
            crow_sb = wp.tile([128, E], F32, tag="crow")
            nc.sync.dma_start(crow_sb, crow_d.ap().to_broadcast([128, E]))
            ones_col = wp.tile([128, 1], F32, tag="ones_col")
            nc.vector.memset(ones_col, 1.0)

            A_bf = tabs[:, 0:HID]
            B_bf = tabs[:, HID:2 * HID]

            # ===== persistent gate-phase tiles =====
            lgbf = wp.tile([E, 8, TC], F32, tag="lgbf")
            lgt_all = psg.tile([128, NB * E], F32, tag="misc")
            hh_sb = wp.tile([128, 8, TC], BF16, tag="hh")
            sgall = wp.tile([128, 8, TC], F32, tag="sgall")
            oh1 = gp.tile([128, NB * E], F32, tag="oh1all")
            oh1v = oh1.rearrange("p (b e) -> p b e", e=E)
            oh2 = gp.tile([128, NB * E], F32, tag="oh2all")
            oh2v = oh2.rearrange("p (b e) -> p b e", e=E)
            wt1 = gp.tile([128, NB], F32, tag="wt1all")
            wt2 = gp.tile([128, NB], F32, tag="wt2all")

            def gate_chunk(j):
                xj = x8a_sb[:, j, :].rearrange("p (k t) -> p k t", t=TC)
                lgT = psl.tile([2 * E, TC], F32, tag="lgT")
                for q in range(4):
                    nc.tensor.matmul(lgT, lhsT=wg8_sb[:, 2 * q:2 * q + 2, :],
                                     rhs=xj[:, 2 * q:2 * q + 2, :],
                                     start=(q == 0), stop=(q == 3), perf_mode=DR)
                nc.vector.tensor_copy(lgbf[:, j, :], lgT[0:E, :])
                for q in range(4):
                    nc.tensor.transpose(lgt_all[:, ts(4 * j + q, E)],
                                        lgbf[:, j, ts(q, 128)], id8f_sb)

            def ffn_h1(J):
                h1 = ps1.tile([128, TC], F32, tag="hsh")
                for kt in range(8):
                    nc.tensor.matmul(h1, lhsT=sw1t_sb[:, kt, ts(J, 128)],
                                     rhs=xtb_sb[:, kt, :],
                                     start=(kt == 0), stop=(kt == 7))
                nc.scalar.activation(sgall[:, J, :], h1, AF.Silu)

            def ffn_h3(J):
                h3 = ps3.tile([128, TC], F32, tag="h3")
                for kt in range(8):
                    nc.tensor.matmul(h3, lhsT=sw3t_sb[:, kt, ts(J, 128)],
                                     rhs=xtb_sb[:, kt, :],
                                     start=(kt == 0), stop=(kt == 7))
                nc.vector.tensor_mul(hh_sb[:, J, :], sgall[:, J, :], h3)

            # ---- incremental gate DVE pass over 128-token blocks [b0, b0+nb)
            def dve_pass(b0, nb):
                lgv = lgt_all.rearrange("p (b e) -> p b e", e=E)[:, b0:b0 + nb, :]

                def bc8(col):
                    return col.unsqueeze(2).to_broadcast([128, nb, E])

                def bc2(col):
                    return col.unsqueeze(3).to_broadcast([128, nb, G, 2])

                mx = wk.tile([128, nb], F32, tag="mx")
                nc.vector.reduce_max(mx, lgv, axis=X)
                sub = wk.tile([128, nb, E], F32, tag="sub")
                nc.vector.tensor_sub(sub, lgv, bc8(mx))
                ex = wk.tile([128, nb, E], F32, tag="ex")
                nc.scalar.activation(ex, sub, AF.Exp, scale=1.0 / GS)
                sm = wk.tile([128, nb], F32, tag="sm")
                nc.vector.reduce_sum(sm, ex, axis=X)
                rcp = wk.tile([128, nb], F32, tag="rcp")
                nc.vector.reciprocal(rcp, sm)
                scores = wk.tile([128, nb, E], F32, tag="scores")
                nc.vector.tensor_mul(scores, ex, bc8(rcp))
                s = wk.tile([128, nb, E], F32, tag="s")
                nc.vector.tensor_add(s, scores,
                                     bias_sb.unsqueeze(1).to_broadcast([128, nb, E]))
                sv = s.rearrange("p b (g two) -> p b g two", two=2)
                g4 = wk.tile([128, nb, G], F32, tag="g4")
                nc.vector.tensor_add(g4, sv[:, :, :, 0], sv[:, :, :, 1])
                gmax = wk.tile([128, nb], F32, tag="gmax")
                nc.vector.reduce_max(gmax, g4, axis=X)
                ohg1 = wk.tile([128, nb, G], F32, tag="ohg1")
                nc.vector.tensor_tensor(ohg1, g4, bc8(gmax)[:, :, 0:G], op=ALU.is_equal)
                gt = wk.tile([128, nb, G], F32, tag="gt")
                nc.vector.tensor_scalar_mul(gt, ohg1, BIG)
                g2 = wk.tile([128, nb, G], F32, tag="g2")
                nc.vector.tensor_sub(g2, g4, gt)
                gmax2 = wk.tile([128, nb], F32, tag="gmax2")
                nc.vector.reduce_max(gmax2, g2, axis=X)
                ohg2 = wk.tile([128, nb, G], F32, tag="ohg2")
                nc.vector.tensor_tensor(ohg2, g2, bc8(gmax2)[:, :, 0:G],
                                        op=ALU.is_equal)
                keep = wk.tile([128, nb, G], F32, tag="keep")
                nc.vector.tensor_add(keep, ohg1, ohg2)
                mk = wk.tile([128, nb, G], F32, tag="mk")
                nc.vector.tensor_scalar(mk, keep, BIG, BIG,
                                        op0=ALU.mult, op1=ALU.subtract)
                # masked = s*keep + (keep*BIG - BIG)   (exact select)
                m0 = wk.tile([128, nb, G, 2], F32, tag="m0")
                nc.vector.tensor_mul(m0, sv, bc2(keep))
                masked = wk.tile([128, nb, G, 2], F32, tag="masked")
                nc.vector.tensor_add(masked, m0, bc2(mk))
                maskedv = masked.rearrange("p b g two -> p b (g two)")
                m1 = wk.tile([128, nb], F32, tag="m1")
                nc.vector.reduce_max(m1, maskedv, axis=X)
                o1 = oh1v[:, b0:b0 + nb, :]
                nc.vector.tensor_tensor(o1, maskedv, bc8(m1), op=ALU.is_equal)
                t2 = wk.tile([128, nb, E], F32, tag="t2")
                nc.vector.tensor_scalar_mul(t2, o1, BIG)
                masked2 = wk.tile([128, nb, E], F32, tag="masked2")
                nc.vector.tensor_sub(masked2, maskedv, t2)
                m2 = wk.tile([128, nb], F32, tag="m2")
                nc.vector.reduce_max(m2, masked2, axis=X)
                o2 = oh2v[:, b0:b0 + nb, :]
                nc.vector.tensor_tensor(o2, masked2, bc8(m2), op=ALU.is_equal)
                tw1 = wk.tile([128, nb, E], F32, tag="tw1")
                nc.vector.tensor_mul(tw1, o1, scores)
                nc.vector.reduce_sum(wt1[:, b0:b0 + nb], tw1, axis=X)
                tw2 = wk.tile([128, nb, E], F32, tag="tw2")
                nc.vector.tensor_mul(tw2, o2, scores)
                nc.vector.reduce_sum(wt2[:, b0:b0 + nb], tw2, axis=X)

            # ===== statically interleaved PE stream, ordered by DMA arrival
            sched = [('1', 0), ('1', 1), ('g', 0), ('1', 2), ('g', 1), ('1', 3),
                     ('g', 2), ('1', 4), ('g', 3), ('1', 5), ('3', 0), ('1', 6),
                     ('3', 1), ('1', 7), ('3', 2), ('g', 4), ('3', 3), ('g', 5),
                     ('3', 4), ('g', 6), ('3', 5), ('g', 7), ('3', 6), ('3', 7)]
            gates_done = 0
            for kind, idx in sched:
                if kind == 'g':
                    gate_chunk(idx)
                    gates_done += 1
                    if gates_done % 2 == 0:
                        dve_pass(8 * (gates_done // 2 - 1), 8)
                elif kind == '1':
                    ffn_h1(idx)
                else:
                    ffn_h3(idx)

            # ===== global counts -> -offsets, all local =====
            ohs = wk.tile([128, NB, E], F32, tag="ohs")
            nc.vector.tensor_add(ohs, oh1v, oh2v)
            fold = NB
            while fold > 1:
                fold //= 2
                nc.vector.tensor_add(ohs[:, 0:fold, :], ohs[:, 0:fold, :],
                                      ohs[:, fold:2 * fold, :])
            cnt_ps = psg.tile([E, 1], F32, tag="misc")
            nc.tensor.matmul(cnt_ps, lhsT=ohs[:, 0, :], rhs=ones_col,
                             start=True, stop=True)
            cnt_sb = wk.tile([E, 1], F32, tag="cntsb")
            nc.scalar.copy(cnt_sb, cnt_ps)
            # noffs[p, e] = -inclusive_cumsum(cnt)[e], broadcast over partitions
            rhs8 = wk.tile([E, E], F32, tag="rhs8")
            nc.vector.tensor_scalar_mul(rhs8, negL8_sb, cnt_sb)
            noffs_ps = psg.tile([128, E], F32, tag="misc")
            nc.tensor.matmul(noffs_ps, lhsT=ones8_sb, rhs=rhs8, start=True, stop=True)
            noffs = wp.tile([128, E], F32, tag="noffs")
            nc.vector.tensor_copy(noffs, noffs_ps)

            # ===== phi row-sets interleaved with FFN output GEMM tiles =====
            def out_tile(Dt):
                sh = ps1.tile([128, TC], F32, tag="hsh")
                for J in range(8):
                    nc.tensor.matmul(sh, lhsT=sw2t_sb[:, J, ts(Dt, 128)],
                                     rhs=hh_sb[:, J, :],
                                     start=(J == 0), stop=(J == 7))
                o_sb = wk.tile([128, TC], BF16, tag="osbt")
                nc.scalar.copy(o_sb, sh)
                nc.sync.dma_start(out.ap()[ts(Dt, 128), :], o_sb)

            # batched stage-A: one-hot (segment x chosen-expert) masks for all
            # 8 row-sets in a handful of wide DVE ops
            ivJ8 = wk.tile([128, E], F32, tag="ivJ8")   # global row idx per rowset
            nc.vector.tensor_add(ivJ8, ivec_sb.to_broadcast([128, E]), crow_sb)
            Gsum = wk.tile([128, E, E], F32, tag="Gsum")   # [p, rs, e]
            nc.vector.tensor_tensor(Gsum, ivJ8.unsqueeze(2).to_broadcast([128, E, E]),
                                    noffs.unsqueeze(1).to_broadcast([128, E, E]),
                                    op=ALU.add)
            Gm8 = wk.tile([128, E, E], F32, tag="Gm8")
            nc.vector.tensor_scalar(Gm8, Gsum, 0.0, 0.0, op0=ALU.add, op1=ALU.is_ge)
            osb8 = wk.tile([128, E, E], F32, tag="osb8")
            nc.vector.tensor_sub(osb8[:, :, 1:E], Gm8[:, :, 0:E - 1], Gm8[:, :, 1:E])
            nc.vector.tensor_scalar(osb8[:, :, 0:1], Gm8[:, :, 0:1], -1.0, 1.0,
                                    op0=ALU.mult, op1=ALU.add)
            osb8v = osb8.rearrange("p (J k) e -> p J k e", k=2)
            ote8 = []
            for k in range(2):
                ohv = (oh1v if k == 0 else oh2v)
                o8 = gp.tile([128, 4, E * E], BF16, tag=f"ote8k{k}")
                o8v = o8.rearrange("p J (e t) -> p J e t", t=E)
                nc.vector.tensor_tensor(
                    o8v,
                    osb8v[:, :, k, :].unsqueeze(3).to_broadcast([128, 4, E, E]),
                    ohv[:, 0:4, :].unsqueeze(2).to_broadcast([128, 4, E, E]),
                    op=ALU.mult)
                ote8.append(o8)

            H_ps = psh.tile([E * E, HID], F32, tag="acc")
            for Jb in range(4):
                for k in range(2):
                    rs_i = Jb * 2 + k
                    wtk = (wt1 if k == 0 else wt2)[:, Jb:Jb + 1]
                    ote = ote8[k][:, Jb, :]
                    otT_ps = psg.tile([E * E, 128], BF16, tag="misc")
                    nc.tensor.transpose(otT_ps, ote, idbf_sb)
                    otT = wk.tile([E * E, 128], BF16, tag="otTsb")
                    nc.vector.tensor_copy(otT, otT_ps)
                    a_ps = ps3.tile([128, HID], F32, tag="h3")
                    nc.tensor.matmul(a_ps, lhsT=otT, rhs=A_bf, start=True, stop=True)
                    b_ps = ps3.tile([128, HID], F32, tag="h3")
                    nc.tensor.matmul(b_ps, lhsT=otT, rhs=B_bf, start=True, stop=True)
                    # phi = silu(w*a) * (w*b)
                    sg = wk.tile([128, HID], F32, tag="phia")
                    nc.scalar.activation(sg, a_ps, AF.Silu, scale=wtk)
                    phi = gp.tile([128, HID], BF16, tag=f"phi{rs_i}")
                    nc.vector.scalar_tensor_tensor(phi, b_ps, wtk, sg,
                                                   op0=ALU.mult, op1=ALU.mult)
                    nc.tensor.matmul(H_ps, lhsT=ote, rhs=phi,
                                     start=(rs_i == 0), stop=(rs_i == 7))
                    out_tile(rs_i)
            H_sb = wk.tile([E * E, HID], F32, tag="Hsb")
            nc.vector.tensor_copy(H_sb, H_ps)
            nc.sync.dma_start(hout.ap(), H_sb)

    nc.compile()
    return nc


_NC = None


def _get_nc():
    global _NC
    if _NC is None:
        _NC = build()
    return _NC


def _pack(a, k):
    """[k*128, f] -> [128, k, f] partition-major contiguous."""
    kk, f = a.shape
    assert kk == k * 128
    return np.ascontiguousarray(a.reshape(k, 128, f).transpose(1, 0, 2))


def make_in_maps(x, w_gate, w1, w2, w3, sw1, sw2, sw3, expert_bias):
    bf = ml_dtypes.bfloat16
    f8 = ml_dtypes.float8_e4m3fn
    xf = np.ascontiguousarray(np.asarray(x, np.float32).reshape(NTOK, D))
    xT = np.ascontiguousarray(xf.T)                       # [D, NTOK]
    wgp = np.zeros((D, 2 * E), np.float32)
    wgp[:, :E] = np.asarray(w_gate, np.float32).T * GS
    wg8_np = _pack(wgp.astype(f8), 8)
    sw1t_np = _pack(np.ascontiguousarray(np.asarray(sw1, np.float32).T).astype(bf), 8)
    sw3t_np = _pack(np.ascontiguousarray(np.asarray(sw3, np.float32).T).astype(bf), 8)
    sw2t_np = _pack(np.ascontiguousarray(np.asarray(sw2, np.float32).T).astype(bf), 8)
    bias_np = np.ascontiguousarray(np.asarray(expert_bias, np.float32).reshape(1, E))
    # host tables: A[8e+t] = x[t] @ w1[e], B likewise with w3   [64, 512] each
    w1_np = np.asarray(w1, np.float32)
    w3_np = np.asarray(w3, np.float32)
    x8 = xf[:E]                                           # [8, D]
    A = np.einsum('td,edh->eth', x8, w1_np).reshape(E * E, HID)
    B = np.einsum('td,edh->eth', x8, w3_np).reshape(E * E, HID)
    tabs_np = np.ascontiguousarray(
        np.concatenate([A, B], axis=1).astype(bf))        # [64, 1024]
    # per-core x: own 512-token block first, then the other blocks in order
    xt8_pk = _pack(xT.astype(f8), 8)                      # [128, 8, NTOK] fp8
    xt_pk = _pack(xT.astype(bf), 8)                       # [128, 8, NTOK] bf16
    in_maps = []
    for c in range(C):
        order = [c] + [j for j in range(8) if j != c]
        x8a_np = np.ascontiguousarray(
            xt8_pk.reshape(128, 8, 8, TC)[:, :, order, :].transpose(0, 2, 1, 3)
            .reshape(128, 8, 8 * TC))
        xtb_np = np.ascontiguousarray(xt_pk[:, :, c * TC:(c + 1) * TC])
        in_maps.append({
            "x8a": x8a_np,
            "xtb": xtb_np,
            "wg8": wg8_np,
            "sw1t": sw1t_np,
            "sw3t": sw3t_np,
            "sw2t": sw2t_np,
            "tabs": tabs_np,
            "biasd": bias_np,
            "ivec": (1024.0 * c + 2.0 * np.arange(128, dtype=np.float32)).reshape(128, 1),
        })
    return in_maps


def combine_outputs(results, w2):
    full = np.empty((NTOK, D), np.float32)
    Hsum = np.zeros((E * E, HID), np.float32)
    for c in range(C):
        full[c * TC:(c + 1) * TC] = results[c]["out"].T.astype(np.float32)
        Hsum += results[c]["hout"]
    # delta[t] = sum_e H[8e+t] @ w2[e]   (t-major rows @ stacked w2)
    Ht = Hsum.reshape(E, E, HID).transpose(1, 0, 2).reshape(E, E * HID)
    delta = Ht @ np.asarray(w2, np.float32).reshape(E * HID, D)
    full[:E] += delta
    return full.reshape(2, 2048, D)


def kernel(x, w_gate, w1, w2, w3, sw1, sw2, sw3, expert_bias, **_unused):
    nc = _get_nc()
    in_maps = make_in_maps(x, w_gate, w1, w2, w3, sw1, sw2, sw3, expert_bias)
    res = bass_utils.run_bass_kernel_spmd(nc, in_maps, core_ids=list(range(C)))
    return combine_outputs(res.results, w2)


# revision 3
# speedup vs baseline: 1.1534x; 1.1534x over previous
"""Trainium2 Bass kernel for grouped-top-k MoE with shared expert (8 NeuronCores, SPMD).

Zero-collective design
----------------------
The reference's "dispatch" gathers rows of x by *expert id* (values 0..7), so the
routed path only ever reads x[0:8] and scatter-adds into output rows 0..7.  Writing
routed_out row i as g(w_i * x[t_i]; e_i) with t_i = chosen expert of assignment i and
e_i = ragged-segment expert of global row i, the whole routed computation factors
through a 64-row table:
    a[t,e] = x[t] @ w1[e],  b[t,e] = x[t] @ w3[e]            (host precompute)
    H[t,e] = sum_{i: t_i=t, e_i=e} silu(w_i*a[t,e]) * (w_i*b[t,e])
    delta[t] = sum_e H[t,e] @ w2[e];   out[t] += delta[t]  (t < 8, host combine)

No collectives at all (v1's three collectives cost 75us + a 42us rendezvous
barrier for <160KB of payload):
  - every core computes the GATE for all 4096 tokens, so global expert
    counts/offsets are available locally.  The gate matmuls run in fp8
    (DoubleRow, 2x) on a 16x-prescaled w_gate; the softmax Exp rescales.
    Routing differs from the f32 reference on ~160/4096 near-tie tokens, which
    only perturbs the 8 delta rows (measured ~0.009 total rel err vs 2e-2 gate).
  - the 64x512 a/b tables are computed on host during input packing and DMA'd.
  - each core emits its partial H [64,512] f32; the host sums the 8 partials
    and applies the tiny w2 GEMM in f32 during unshard (the v1 kernel already
    host-summed partial deltas).
  - data-parallel over tokens for the shared-expert FFN (512 tokens/core);
    per-core x is packed with the OWN 512-token block first so one NEFF serves
    all cores.

Scheduling: one statically interleaved PE stream ordered by DMA arrival -
h1 tiles (sw1t) first with gate chunks riding the fp8 x blocks, h3 tiles after
sw3t, alternating tags so PSUM rotations never stall; the gate DVE chain runs
in 4 incremental passes so counts are ready right after the last gate chunk;
phi/H then out-GEMMs form the tail.
"""

import sys

if "/opt/trn_rl_repo" not in sys.path:
    sys.path.insert(0, "/opt/trn_rl_repo")

import numpy as np
import ml_dtypes

import concourse.bass as bass
import concourse.mybir as mybir
import concourse.tile as tile
from concourse import bacc
from concourse import bass_utils

F32 = mybir.dt.float32
BF16 = mybir.dt.bfloat16
FP8 = mybir.dt.float8e4
DR = mybir.MatmulPerfMode.DoubleRow
GS = 16.0   # gate fp8 weight pre-scale (softmax Exp divides it back out)
AF = mybir.ActivationFunctionType
ALU = mybir.AluOpType
X = mybir.AxisListType.X

E = 8          # experts (== table token count == cores)
G = 4          # expert groups
D = 1024       # model dim
HID = 512      # expert hidden
SH = 1024      # shared-expert hidden
C = 8          # cores
TC = 512       # tokens per core
NTOK = 4096
NB = 32        # 128-token blocks globally
BIG = 1.0e30


def ts(i, s):
    return slice(i * s, (i + 1) * s)


def build():
    nc = bacc.Bacc("TRN2", target_bir_lowering=False, debug=False, num_devices=C)

    # ---- I/O: packed partition-major; contraction dim = k*128+p
    wg8 = nc.dram_tensor("wg8", [128, 8, 2 * E], FP8, kind="ExternalInput")
    biasd = nc.dram_tensor("biasd", [1, E], F32, kind="ExternalInput")
    ivec = nc.dram_tensor("ivec", [128, 1], F32, kind="ExternalInput")
    # all 4096 tokens in fp8 (gate only), dim-major; block 0 = own shard
    x8a = nc.dram_tensor("x8a", [128, 8, 8 * TC], FP8, kind="ExternalInput")  # block-major
    xtb = nc.dram_tensor("xtb", [128, 8, TC], BF16, kind="ExternalInput")
    sw1t = nc.dram_tensor("sw1t", [128, 8, SH], BF16, kind="ExternalInput")
    sw3t = nc.dram_tensor("sw3t", [128, 8, SH], BF16, kind="ExternalInput")
    sw2t = nc.dram_tensor("sw2t", [128, 8, D], BF16, kind="ExternalInput")
    tabs_d = nc.dram_tensor("tabs", [E * E, 2 * HID], BF16, kind="ExternalInput")
    out = nc.dram_tensor("out", [D, TC], BF16, kind="ExternalOutput")  # shared^T shard
    hout = nc.dram_tensor("hout", [E * E, HID], F32, kind="ExternalOutput")  # partial H

    # ---- compile-time constants (embedded in NEFF)
    idbf_d = nc.inline_tensor(np.eye(128, dtype=ml_dtypes.bfloat16), name="idbf")
    id8f_d = nc.inline_tensor(np.eye(E, dtype=np.float32), name="id8f")
    # negL8[k, e] = -1 if k <= e else 0;  noffs[e] = sum_k negL8[k,e]*cnt[k]
    negL8_d = nc.inline_tensor(
        np.ascontiguousarray(-np.tril(np.ones((E, E), np.float32)).T), name="negL8")
    ones8_d = nc.inline_tensor(np.ones((E, 128), np.float32), name="ones8x128")
    crow_d = nc.inline_tensor(
        np.array([[256 * Jb + k for Jb in range(4) for k in range(2)]],
                 np.float32).reshape(1, E), name="crow")

    with tile.TileContext(nc) as tc:
        with (
            tc.tile_pool(name="wp", bufs=1) as wp,       # persistent SBUF
            tc.tile_pool(name="gp", bufs=1) as gp,       # gate outputs (persist to phi)
            tc.tile_pool(name="wk", bufs=2) as wk,       # transient SBUF
            tc.tile_pool(name="psg", bufs=1, space="PSUM") as psg,   # transposes/misc
            tc.tile_pool(name="psl", bufs=1, space="PSUM") as psl,   # gate logit chunks
            tc.tile_pool(name="psh", bufs=1, space="PSUM") as psh,   # H accumulator
            tc.tile_pool(name="ps1", bufs=3, space="PSUM") as ps1,   # h1 + out tiles
            tc.tile_pool(name="ps3", bufs=2, space="PSUM") as ps3,   # h3 + phi a/b
        ):
            # ===== loads, ordered by first use (kt-halves so tiles start early)
            wg8_sb = wp.tile([128, 8, 2 * E], FP8, tag="wg8")
            nc.sync.dma_start(wg8_sb, wg8.ap())
            x8a_sb = wp.tile([128, 8, 8 * TC], FP8, tag="x8a")  # [p, blk, kt*tok]
            nc.sync.dma_start(x8a_sb[:, 0, :], x8a.ap()[:, 0, :])
            id8f_sb = wp.tile([E, E], F32, tag="id8f")
            nc.sync.dma_start(id8f_sb, id8f_d.ap())
            sw1t_sb = wp.tile([128, 8, SH], BF16, tag="sw1t")
            xtb_sb = wp.tile([128, 8, TC], BF16, tag="xtb")
            for q in range(4):
                nc.sync.dma_start(sw1t_sb[:, 2 * q:2 * q + 2, :],
                                  sw1t.ap()[:, 2 * q:2 * q + 2, :])
                nc.sync.dma_start(xtb_sb[:, 2 * q:2 * q + 2, :],
                                  xtb.ap()[:, 2 * q:2 * q + 2, :])
            bias_sb = wp.tile([128, E], F32, tag="bias")
            nc.sync.dma_start(bias_sb, biasd.ap().to_broadcast([128, E]))
            for j in range(1, 4):
                nc.sync.dma_start(x8a_sb[:, j, :], x8a.ap()[:, j, :])
            sw3t_sb = wp.tile([128, 8, SH], BF16, tag="sw3t")
            nc.sync.dma_start(sw3t_sb[:, 0:4, :], sw3t.ap()[:, 0:4, :])
            for j in range(4, 6):
                nc.sync.dma_start(x8a_sb[:, j, :], x8a.ap()[:, j, :])
            nc.sync.dma_start(sw3t_sb[:, 4:8, :], sw3t.ap()[:, 4:8, :])
            for j in range(6, 8):
                nc.sync.dma_start(x8a_sb[:, j, :], x8a.ap()[:, j, :])
            tabs = wp.tile([E * E, 2 * HID], BF16, tag="tabs")
            nc.sync.dma_start(tabs, tabs_d.ap())
            sw2t_sb = wp.tile([128, 8, D], BF16, tag="sw2t")
            nc.sync.dma_start(sw2t_sb, sw2t.ap())
            # small late-use constants
            ivec_sb = wp.tile([128, 1], F32, tag="ivec")
            nc.sync.dma_start(ivec_sb, ivec.ap())
            idbf_sb = wp.tile([128, 128], BF16, tag="idbf")
            nc.sync.dma_start(idbf_sb, idbf_d.ap())
            negL8_sb = wp.tile([E, E], F32, tag="negL8")
            nc.sync.dma_start(negL8_sb, negL8_d.ap())
            ones8_sb = wp.tile([E, 128], F32, tag="ones8")
            nc.sync.dma_start(ones8_sb, ones8_d.ap())
            crow_sb = wp.tile([128, E], F32, tag="crow")
            nc.sync.dma_start(crow_sb, crow_d.ap().to_broadcast([128, E]))
            ones_col = wp.tile([128, 1], F32, tag="ones_col")
            nc.vector.memset(ones_col, 1.0)

            A_bf = tabs[:, 0:HID]
            B_bf = tabs[:, HID:2 * HID]

            # ===== persistent gate-phase tiles =====
            lgbf = wp.tile([E, 8, TC], F32, tag="lgbf")
            lgt_all = psg.tile([128, NB * E], F32, tag="misc")
            hh_sb = wp.tile([128, 8, TC], BF16, tag="hh")
            sgall = wp.tile([128, 8, TC], F32, tag="sgall")
            oh1 = gp.tile([128, NB * E], F32, tag="oh1all")
            oh1v = oh1.rearrange("p (b e) -> p b e", e=E)
            oh2 = gp.tile([128, NB * E], F32, tag="oh2all")
            oh2v = oh2.rearrange("p (b e) -> p b e", e=E)
            wt1 = gp.tile([128, NB], F32, tag="wt1all")
            wt2 = gp.tile([128, NB], F32, tag="wt2all")

            def gate_chunk(j):
                xj = x8a_sb[:, j, :].rearrange("p (k t) -> p k t", t=TC)
                lgT = psl.tile([2 * E, TC], F32, tag="lgT")
                for q in range(4):
                    nc.tensor.matmul(lgT, lhsT=wg8_sb[:, 2 * q:2 * q + 2, :],
                                     rhs=xj[:, 2 * q:2 * q + 2, :],
                                     start=(q == 0), stop=(q == 3), perf_mode=DR)
                nc.vector.tensor_copy(lgbf[:, j, :], lgT[0:E, :])
                for q in range(4):
                    nc.tensor.transpose(lgt_all[:, ts(4 * j + q, E)],
                                        lgbf[:, j, ts(q, 128)], id8f_sb)

            def ffn_h1(J):
                h1 = ps1.tile([128, TC], F32, tag="hsh")
                for kt in range(8):
                    nc.tensor.matmul(h1, lhsT=sw1t_sb[:, kt, ts(J, 128)],
                                     rhs=xtb_sb[:, kt, :],
                                     start=(kt == 0), stop=(kt == 7))
                nc.scalar.activation(sgall[:, J, :], h1, AF.Silu)

            def ffn_h3(J):
                h3 = ps3.tile([128, TC], F32, tag="h3")
                for kt in range(8):
                    nc.tensor.matmul(h3, lhsT=sw3t_sb[:, kt, ts(J, 128)],
                                     rhs=xtb_sb[:, kt, :],
                                     start=(kt == 0), stop=(kt == 7))
                nc.vector.tensor_mul(hh_sb[:, J, :], sgall[:, J, :], h3)

            # ---- incremental gate DVE pass over 128-token blocks [b0, b0+nb)
            def dve_pass(b0, nb):
                lgv = lgt_all.rearrange("p (b e) -> p b e", e=E)[:, b0:b0 + nb, :]

                def bc8(col):
                    return col.unsqueeze(2).to_broadcast([128, nb, E])

                def bc2(col):
                    return col.unsqueeze(3).to_broadcast([128, nb, G, 2])

                mx = wk.tile([128, nb], F32, tag="mx")
                nc.vector.reduce_max(mx, lgv, axis=X)
                sub = wk.tile([128, nb, E], F32, tag="sub")
                nc.vector.tensor_sub(sub, lgv, bc8(mx))
                ex = wk.tile([128, nb, E], F32, tag="ex")
                nc.scalar.activation(ex, sub, AF.Exp, scale=1.0 / GS)
                sm = wk.tile([128, nb], F32, tag="sm")
                nc.vector.reduce_sum(sm, ex, axis=X)
                rcp = wk.tile([128, nb], F32, tag="rcp")
                nc.vector.reciprocal(rcp, sm)
                scores = wk.tile([128, nb, E], F32, tag="scores")
                nc.vector.tensor_mul(scores, ex, bc8(rcp))
                s = wk.tile([128, nb, E], F32, tag="s")
                nc.vector.tensor_add(s, scores,
                                     bias_sb.unsqueeze(1).to_broadcast([128, nb, E]))
                sv = s.rearrange("p b (g two) -> p b g two", two=2)
                g4 = wk.tile([128, nb, G], F32, tag="g4")
                nc.vector.tensor_add(g4, sv[:, :, :, 0], sv[:, :, :, 1])
                gmax = wk.tile([128, nb], F32, tag="gmax")
                nc.vector.reduce_max(gmax, g4, axis=X)
                ohg1 = wk.tile([128, nb, G], F32, tag="ohg1")
                nc.vector.tensor_tensor(ohg1, g4, bc8(gmax)[:, :, 0:G], op=ALU.is_equal)
                gt = wk.tile([128, nb, G], F32, tag="gt")
                nc.vector.tensor_scalar_mul(gt, ohg1, BIG)
                g2 = wk.tile([128, nb, G], F32, tag="g2")
                nc.vector.tensor_sub(g2, g4, gt)
                gmax2 = wk.tile([128, nb], F32, tag="gmax2")
                nc.vector.reduce_max(gmax2, g2, axis=X)
                ohg2 = wk.tile([128, nb, G], F32, tag="ohg2")
                nc.vector.tensor_tensor(ohg2, g2, bc8(gmax2)[:, :, 0:G],
                                        op=ALU.is_equal)
                keep = wk.tile([128, nb, G], F32, tag="keep")
                nc.vector.tensor_add(keep, ohg1, ohg2)
                mk = wk.tile([128, nb, G], F32, tag="mk")
                nc.vector.tensor_scalar(mk, keep, BIG, BIG,
                                        op0=ALU.mult, op1=ALU.subtract)
                # masked = s*keep + (keep*BIG - BIG)   (exact select)
                m0 = wk.tile([128, nb, G, 2], F32, tag="m0")
                nc.vector.tensor_mul(m0, sv, bc2(keep))
                masked = wk.tile([128, nb, G, 2], F32, tag="masked")
                nc.vector.tensor_add(masked, m0, bc2(mk))
                maskedv = masked.rearrange("p b g two -> p b (g two)")
                m1 = wk.tile([128, nb], F32, tag="m1")
                nc.vector.reduce_max(m1, maskedv, axis=X)
                o1 = oh1v[:, b0:b0 + nb, :]
                nc.vector.tensor_tensor(o1, maskedv, bc8(m1), op=ALU.is_equal)
                t2 = wk.tile([128, nb, E], F32, tag="t2")
                nc.vector.tensor_scalar_mul(t2, o1, BIG)
                masked2 = wk.tile([128, nb, E], F32, tag="masked2")
                nc.vector.tensor_sub(masked2, maskedv, t2)
                m2 = wk.tile([128, nb], F32, tag="m2")
                nc.vector.reduce_max(m2, masked2, axis=X)
                o2 = oh2v[:, b0:b0 + nb, :]
                nc.vector.tensor_tensor(o2, masked2, bc8(m2), op=ALU.is_equal)
                tw1 = wk.tile([128, nb, E], F32, tag="tw1")
                nc.vector.tensor_mul(tw1, o1, scores)
                nc.vector.reduce_sum(wt1[:, b0:b0 + nb], tw1, axis=X)
                tw2 = wk.tile([128, nb, E], F32, tag="tw2")
                nc.vector.tensor_mul(tw2, o2, scores)
                nc.vector.reduce_sum(wt2[:, b0:b0 + nb], tw2, axis=X)

            # ===== statically interleaved PE stream, ordered by DMA arrival
            sched = [('g', 0), ('1', 0), ('1', 1), ('1', 2), ('g', 1), ('1', 3),
                     ('g', 2), ('1', 4), ('g', 3), ('1', 5), ('3', 0), ('1', 6),
                     ('3', 1), ('1', 7), ('3', 2), ('g', 4), ('3', 3), ('g', 5),
                     ('3', 4), ('g', 6), ('3', 5), ('g', 7), ('3', 6), ('3', 7)]
            gates_done = 0
            for kind, idx in sched:
                if kind == 'g':
                    gate_chunk(idx)
                    gates_done += 1
                    if gates_done % 2 == 0:
                        dve_pass(8 * (gates_done // 2 - 1), 8)
                elif kind == '1':
                    ffn_h1(idx)
                else:
                    ffn_h3(idx)

            # ===== global counts -> -offsets, all local =====
            ohs = wk.tile([128, NB, E], F32, tag="ohs")
            nc.vector.tensor_add(ohs, oh1v, oh2v)
            fold = NB
            while fold > 1:
                fold //= 2
                nc.vector.tensor_add(ohs[:, 0:fold, :], ohs[:, 0:fold, :],
                                      ohs[:, fold:2 * fold, :])
            cnt_ps = psg.tile([E, 1], F32, tag="misc")
            nc.tensor.matmul(cnt_ps, lhsT=ohs[:, 0, :], rhs=ones_col,
                             start=True, stop=True)
            cnt_sb = wk.tile([E, 1], F32, tag="cntsb")
            nc.scalar.copy(cnt_sb, cnt_ps)
            # noffs[p, e] = -inclusive_cumsum(cnt)[e], broadcast over partitions
            rhs8 = wk.tile([E, E], F32, tag="rhs8")
            nc.vector.tensor_scalar_mul(rhs8, negL8_sb, cnt_sb)
            noffs_ps = psg.tile([128, E], F32, tag="misc")
            nc.tensor.matmul(noffs_ps, lhsT=ones8_sb, rhs=rhs8, start=True, stop=True)
            noffs = wp.tile([128, E], F32, tag="noffs")
            nc.vector.tensor_copy(noffs, noffs_ps)

            # ===== phi row-sets interleaved with FFN output GEMM tiles =====
            def out_tile(Dt):
                sh = ps1.tile([128, TC], F32, tag="hsh")
                for J in range(8):
                    nc.tensor.matmul(sh, lhsT=sw2t_sb[:, J, ts(Dt, 128)],
                                     rhs=hh_sb[:, J, :],
                                     start=(J == 0), stop=(J == 7))
                o_sb = wk.tile([128, TC], BF16, tag="osbt")
                nc.scalar.copy(o_sb, sh)
                nc.sync.dma_start(out.ap()[ts(Dt, 128), :], o_sb)

            # batched stage-A: one-hot (segment x chosen-expert) masks for all
            # 8 row-sets in a handful of wide DVE ops
            ivJ8 = wk.tile([128, E], F32, tag="ivJ8")   # global row idx per rowset
            nc.vector.tensor_add(ivJ8, ivec_sb.to_broadcast([128, E]), crow_sb)
            Gsum = wk.tile([128, E, E], F32, tag="Gsum")   # [p, rs, e]
            nc.vector.tensor_tensor(Gsum, ivJ8.unsqueeze(2).to_broadcast([128, E, E]),
                                    noffs.unsqueeze(1).to_broadcast([128, E, E]),
                                    op=ALU.add)
            Gm8 = wk.tile([128, E, E], F32, tag="Gm8")
            nc.vector.tensor_scalar(Gm8, Gsum, 0.0, 0.0, op0=ALU.add, op1=ALU.is_ge)
            osb8 = wk.tile([128, E, E], F32, tag="osb8")
            nc.vector.tensor_sub(osb8[:, :, 1:E], Gm8[:, :, 0:E - 1], Gm8[:, :, 1:E])
            nc.vector.tensor_scalar(osb8[:, :, 0:1], Gm8[:, :, 0:1], -1.0, 1.0,
                                    op0=ALU.mult, op1=ALU.add)
            osb8v = osb8.rearrange("p (J k) e -> p J k e", k=2)
            ote8 = []
            for k in range(2):
                ohv = (oh1v if k == 0 else oh2v)
                o8 = gp.tile([128, 4, E * E], BF16, tag=f"ote8k{k}")
                o8v = o8.rearrange("p J (e t) -> p J e t", t=E)
                nc.vector.tensor_tensor(
                    o8v,
                    osb8v[:, :, k, :].unsqueeze(3).to_broadcast([128, 4, E, E]),
                    ohv[:, 0:4, :].unsqueeze(2).to_broadcast([128, 4, E, E]),
                    op=ALU.mult)
                ote8.append(o8)

            H_ps = psh.tile([E * E, HID], F32, tag="acc")
            for Jb in range(4):
                for k in range(2):
                    rs_i = Jb * 2 + k
                    wtk = (wt1 if k == 0 else wt2)[:, Jb:Jb + 1]
                    ote = ote8[k][:, Jb, :]
                    otT_ps = psg.tile([E * E, 128], BF16, tag="misc")
                    nc.tensor.transpose(otT_ps, ote, idbf_sb)
                    otT = wk.tile([E * E, 128], BF16, tag="otTsb")
                    nc.vector.tensor_copy(otT, otT_ps)
                    a_ps = ps3.tile([128, HID], F32, tag="h3")
                    nc.tensor.matmul(a_ps, lhsT=otT, rhs=A_bf, start=True, stop=True)
                    b_ps = ps3.tile([128, HID], F32, tag="h3")
                    nc.tensor.matmul(b_ps, lhsT=otT, rhs=B_bf, start=True, stop=True)
                    # phi = silu(w*a) * (w*b)
                    sg = wk.tile([128, HID], F32, tag="phia")
                    nc.scalar.activation(sg, a_ps, AF.Silu, scale=wtk)
                    phi = gp.tile([128, HID], BF16, tag=f"phi{rs_i}")
                    nc.vector.scalar_tensor_tensor(phi, b_ps, wtk, sg,
                                                   op0=ALU.mult, op1=ALU.mult)
                    nc.tensor.matmul(H_ps, lhsT=ote, rhs=phi,
                                     start=(rs_i == 0), stop=(rs_i == 7))
                    out_tile(rs_i)
            H_sb = wk.tile([E * E, HID], F32, tag="Hsb")
            nc.vector.tensor_copy(H_sb, H_ps)
            nc.sync.dma_start(hout.ap(), H_sb)

    nc.compile()
    return nc


_NC = None


def _get_nc():
    global _NC
    if _NC is None:
        _NC = build()
    return _NC


def _pack(a, k):
    """[k*128, f] -> [128, k, f] partition-major contiguous."""
    kk, f = a.shape
    assert kk == k * 128
    return np.ascontiguousarray(a.reshape(k, 128, f).transpose(1, 0, 2))


def make_in_maps(x, w_gate, w1, w2, w3, sw1, sw2, sw3, expert_bias):
    bf = ml_dtypes.bfloat16
    f8 = ml_dtypes.float8_e4m3fn
    xf = np.ascontiguousarray(np.asarray(x, np.float32).reshape(NTOK, D))
    xT = np.ascontiguousarray(xf.T)                       # [D, NTOK]
    wgp = np.zeros((D, 2 * E), np.float32)
    wgp[:, :E] = np.asarray(w_gate, np.float32).T * GS
    wg8_np = _pack(wgp.astype(f8), 8)
    sw1t_np = _pack(np.ascontiguousarray(np.asarray(sw1, np.float32).T).astype(bf), 8)
    sw3t_np = _pack(np.ascontiguousarray(np.asarray(sw3, np.float32).T).astype(bf), 8)
    sw2t_np = _pack(np.ascontiguousarray(np.asarray(sw2, np.float32).T).astype(bf), 8)
    bias_np = np.ascontiguousarray(np.asarray(expert_bias, np.float32).reshape(1, E))
    # host tables: A[8e+t] = x[t] @ w1[e], B likewise with w3   [64, 512] each
    w1_np = np.asarray(w1, np.float32)
    w3_np = np.asarray(w3, np.float32)
    x8 = xf[:E]                                           # [8, D]
    A = np.einsum('td,edh->eth', x8, w1_np).reshape(E * E, HID)
    B = np.einsum('td,edh->eth', x8, w3_np).reshape(E * E, HID)
    tabs_np = np.ascontiguousarray(
        np.concatenate([A, B], axis=1).astype(bf))        # [64, 1024]
    # per-core x: own 512-token block first, then the other blocks in order
    xt8_pk = _pack(xT.astype(f8), 8)                      # [128, 8, NTOK] fp8
    xt_pk = _pack(xT.astype(bf), 8)                       # [128, 8, NTOK] bf16
    in_maps = []
    for c in range(C):
        order = [c] + [j for j in range(8) if j != c]
        x8a_np = np.ascontiguousarray(
            xt8_pk.reshape(128, 8, 8, TC)[:, :, order, :].transpose(0, 2, 1, 3)
            .reshape(128, 8, 8 * TC))
        xtb_np = np.ascontiguousarray(xt_pk[:, :, c * TC:(c + 1) * TC])
        in_maps.append({
            "x8a": x8a_np,
            "xtb": xtb_np,
            "wg8": wg8_np,
            "sw1t": sw1t_np,
            "sw3t": sw3t_np,
            "sw2t": sw2t_np,
            "tabs": tabs_np,
            "biasd": bias_np,
            "ivec": (1024.0 * c + 2.0 * np.arange(128, dtype=np.float32)).reshape(128, 1),
        })
    return in_maps


def combine_outputs(results, w2):
    full = np.empty((NTOK, D), np.float32)
    Hsum = np.zeros((E * E, HID), np.float32)
    for c in range(C):
        full[c * TC:(c + 1) * TC] = results[c]["out"].T.astype(np.float32)
        Hsum += results[c]["hout"]
    # delta[t] = sum_e H[8e+t] @ w2[e]   (t-major rows @ stacked w2)
    Ht = Hsum.reshape(E, E, HID).transpose(1, 0, 2).reshape(E, E * HID)
    delta = Ht @ np.asarray(w2, np.float32).reshape(E * HID, D)
    full[:E] += delta
    return full.reshape(2, 2048, D)


def kernel(x, w_gate, w1, w2, w3, sw1, sw2, sw3, expert_bias, **_unused):
    nc = _get_nc()
    in_maps = make_in_maps(x, w_gate, w1, w2, w3, sw1, sw2, sw3, expert_bias)
    res = bass_utils.run_bass_kernel_spmd(nc, in_maps, core_ids=list(range(C)))
    return combine_outputs(res.results, w2)


# revision 4
# speedup vs baseline: 1.2054x; 1.0451x over previous
"""Trainium2 Bass kernel for grouped-top-k MoE with shared expert (8 NeuronCores, SPMD).

Zero-collective design
----------------------
The reference's "dispatch" gathers rows of x by *expert id* (values 0..7), so the
routed path only ever reads x[0:8] and scatter-adds into output rows 0..7.  Writing
routed_out row i as g(w_i * x[t_i]; e_i) with t_i = chosen expert of assignment i and
e_i = ragged-segment expert of global row i, the whole routed computation factors
through a 64-row table:
    a[t,e] = x[t] @ w1[e],  b[t,e] = x[t] @ w3[e]            (host precompute)
    H[t,e] = sum_{i: t_i=t, e_i=e} silu(w_i*a[t,e]) * (w_i*b[t,e])
    delta[t] = sum_e H[t,e] @ w2[e];   out[t] += delta[t]  (t < 8, host combine)

No collectives at all (v1's three collectives cost 75us + a 42us rendezvous
barrier for <160KB of payload):
  - every core computes the GATE for all 4096 tokens, so global expert
    counts/offsets are available locally.  The gate matmuls run in fp8
    (DoubleRow, 2x) on a 16x-prescaled w_gate; the softmax Exp rescales.
    Routing differs from the f32 reference on ~160/4096 near-tie tokens, which
    only perturbs the 8 delta rows (measured ~0.009 total rel err vs 2e-2 gate).
  - the 64x512 a/b tables are computed on host during input packing and DMA'd.
  - each core emits its partial H [64,512] f32; the host sums the 8 partials
    and applies the tiny w2 GEMM in f32 during unshard (the v1 kernel already
    host-summed partial deltas).
  - data-parallel over tokens for the shared-expert FFN (512 tokens/core);
    per-core x is packed with the OWN 512-token block first so one NEFF serves
    all cores.

Scheduling: one statically interleaved PE stream ordered by DMA arrival -
h1 tiles (sw1t) first with gate chunks riding the fp8 x blocks, h3 tiles after
sw3t, alternating tags so PSUM rotations never stall; the gate DVE chain runs
in 4 incremental passes so counts are ready right after the last gate chunk;
phi/H then out-GEMMs form the tail.
"""

import sys

if "/opt/trn_rl_repo" not in sys.path:
    sys.path.insert(0, "/opt/trn_rl_repo")

import numpy as np
import ml_dtypes

import concourse.bass as bass
import concourse.mybir as mybir
import concourse.tile as tile
from concourse import bacc
from concourse import bass_utils

F32 = mybir.dt.float32
BF16 = mybir.dt.bfloat16
FP8 = mybir.dt.float8e4
DR = mybir.MatmulPerfMode.DoubleRow
GS = 16.0   # gate fp8 weight pre-scale (softmax Exp divides it back out)
AF = mybir.ActivationFunctionType
ALU = mybir.AluOpType
X = mybir.AxisListType.X

E = 8          # experts (== table token count == cores)
G = 4          # expert groups
D = 1024       # model dim
HID = 512      # expert hidden
SH = 1024      # shared-expert hidden
C = 8          # cores
TC = 512       # tokens per core
NTOK = 4096
NB = 32        # 128-token blocks globally
BIG = 1.0e30


def ts(i, s):
    return slice(i * s, (i + 1) * s)


def build():
    nc = bacc.Bacc("TRN2", target_bir_lowering=False, debug=False, num_devices=C)

    # ---- I/O: packed partition-major; contraction dim = k*128+p
    wg8 = nc.dram_tensor("wg8", [128, 8, 2 * E], FP8, kind="ExternalInput")
    biasd = nc.dram_tensor("biasd", [1, E], F32, kind="ExternalInput")
    ivec = nc.dram_tensor("ivec", [128, 1], F32, kind="ExternalInput")
    # all 4096 tokens in fp8 (gate only), dim-major; block 0 = own shard
    x8a = nc.dram_tensor("x8a", [128, 8, 8 * TC], FP8, kind="ExternalInput")  # block-major
    xtb = nc.dram_tensor("xtb", [128, 8, TC], BF16, kind="ExternalInput")
    sw1t = nc.dram_tensor("sw1t", [128, 8, SH], BF16, kind="ExternalInput")
    sw3t = nc.dram_tensor("sw3t", [128, 8, SH], BF16, kind="ExternalInput")
    sw2t = nc.dram_tensor("sw2t", [128, 8, D], BF16, kind="ExternalInput")
    tabs_d = nc.dram_tensor("tabs", [E * E, 2 * HID], BF16, kind="ExternalInput")
    out = nc.dram_tensor("out", [D, TC], BF16, kind="ExternalOutput")  # shared^T shard
    hout = nc.dram_tensor("hout", [E * E, HID], F32, kind="ExternalOutput")  # partial H

    # ---- compile-time constants (embedded in NEFF)
    idbf_d = nc.inline_tensor(np.eye(128, dtype=ml_dtypes.bfloat16), name="idbf")
    id8f_d = nc.inline_tensor(np.eye(E, dtype=np.float32), name="id8f")
    # negL8[k, e] = -1 if k <= e else 0;  noffs[e] = sum_k negL8[k,e]*cnt[k]
    negL8_d = nc.inline_tensor(
        np.ascontiguousarray(-np.tril(np.ones((E, E), np.float32)).T), name="negL8")
    ones8_d = nc.inline_tensor(np.ones((E, 128), np.float32), name="ones8x128")
    crow_d = nc.inline_tensor(
        np.array([[256 * Jb + k for Jb in range(4) for k in range(2)]],
                 np.float32).reshape(1, E), name="crow")

    with tile.TileContext(nc) as tc:
        with (
            tc.tile_pool(name="wp", bufs=1) as wp,       # persistent SBUF
            tc.tile_pool(name="gp", bufs=1) as gp,       # gate outputs (persist to phi)
            tc.tile_pool(name="wk", bufs=2) as wk,       # transient SBUF
            tc.tile_pool(name="psg", bufs=1, space="PSUM") as psg,   # transposes/misc
            tc.tile_pool(name="psl", bufs=1, space="PSUM") as psl,   # gate logit chunks
            tc.tile_pool(name="psh", bufs=1, space="PSUM") as psh,   # H accumulator
            tc.tile_pool(name="ps1", bufs=3, space="PSUM") as ps1,   # h1 + out tiles
            tc.tile_pool(name="ps3", bufs=2, space="PSUM") as ps3,   # h3 + phi a/b
        ):
            # ===== loads, ordered by first use (kt-halves so tiles start early)
            wg8_sb = wp.tile([128, 8, 2 * E], FP8, tag="wg8")
            nc.sync.dma_start(wg8_sb, wg8.ap())
            x8a_sb = wp.tile([128, 8, 8 * TC], FP8, tag="x8a")  # [p, blk, kt*tok]
            nc.sync.dma_start(x8a_sb[:, 0, :], x8a.ap()[:, 0, :])
            id8f_sb = wp.tile([E, E], F32, tag="id8f")
            nc.sync.dma_start(id8f_sb, id8f_d.ap())
            sw1t_sb = wp.tile([128, 8, SH], BF16, tag="sw1t")
            xtb_sb = wp.tile([128, 8, TC], BF16, tag="xtb")
            for q in range(4):
                nc.sync.dma_start(sw1t_sb[:, 2 * q:2 * q + 2, :],
                                  sw1t.ap()[:, 2 * q:2 * q + 2, :])
                nc.sync.dma_start(xtb_sb[:, 2 * q:2 * q + 2, :],
                                  xtb.ap()[:, 2 * q:2 * q + 2, :])
            bias_sb = wp.tile([128, E], F32, tag="bias")
            nc.sync.dma_start(bias_sb, biasd.ap().to_broadcast([128, E]))
            for j in range(1, 4):
                nc.sync.dma_start(x8a_sb[:, j, :], x8a.ap()[:, j, :])
            sw3t_sb = wp.tile([128, 8, SH], BF16, tag="sw3t")
            nc.sync.dma_start(sw3t_sb[:, 0:4, :], sw3t.ap()[:, 0:4, :])
            for j in range(4, 6):
                nc.sync.dma_start(x8a_sb[:, j, :], x8a.ap()[:, j, :])
            nc.sync.dma_start(sw3t_sb[:, 4:8, :], sw3t.ap()[:, 4:8, :])
            for j in range(6, 8):
                nc.sync.dma_start(x8a_sb[:, j, :], x8a.ap()[:, j, :])
            tabs = wp.tile([E * E, 2 * HID], BF16, tag="tabs")
            nc.sync.dma_start(tabs, tabs_d.ap())
            sw2t_sb = wp.tile([128, 8, D], BF16, tag="sw2t")
            nc.sync.dma_start(sw2t_sb, sw2t.ap())
            # small late-use constants
            ivec_sb = wp.tile([128, 1], F32, tag="ivec")
            nc.sync.dma_start(ivec_sb, ivec.ap())
            idbf_sb = wp.tile([128, 128], BF16, tag="idbf")
            nc.sync.dma_start(idbf_sb, idbf_d.ap())
            negL8_sb = wp.tile([E, E], F32, tag="negL8")
            nc.sync.dma_start(negL8_sb, negL8_d.ap())
            ones8_sb = wp.tile([E, 128], F32, tag="ones8")
            nc.sync.dma_start(ones8_sb, ones8_d.ap())
            crow_sb = wp.tile([128, E], F32, tag="crow")
            nc.sync.dma_start(crow_sb, crow_d.ap().to_broadcast([128, E]))
            ones_col = wp.tile([128, 1], F32, tag="ones_col")
            nc.vector.memset(ones_col, 1.0)

            A_bf = tabs[:, 0:HID]
            B_bf = tabs[:, HID:2 * HID]

            # ===== persistent gate-phase tiles =====
            lgbf = wp.tile([E, 8, TC], F32, tag="lgbf")
            lgt_all = psg.tile([128, NB * E], F32, tag="misc")
            hh_sb = wp.tile([128, 8, TC], BF16, tag="hh")
            sgall = wp.tile([128, 8, TC], F32, tag="sgall")
            oh1 = gp.tile([128, NB * E], F32, tag="oh1all")
            oh1v = oh1.rearrange("p (b e) -> p b e", e=E)
            oh2 = gp.tile([128, NB * E], F32, tag="oh2all")
            oh2v = oh2.rearrange("p (b e) -> p b e", e=E)
            wt1 = gp.tile([128, NB], F32, tag="wt1all")
            wt2 = gp.tile([128, NB], F32, tag="wt2all")

            def gate_chunk(j):
                xj = x8a_sb[:, j, :].rearrange("p (k t) -> p k t", t=TC)
                lgT = psl.tile([2 * E, TC], F32, tag="lgT")
                for q in range(4):
                    nc.tensor.matmul(lgT, lhsT=wg8_sb[:, 2 * q:2 * q + 2, :],
                                     rhs=xj[:, 2 * q:2 * q + 2, :],
                                     start=(q == 0), stop=(q == 3), perf_mode=DR)
                nc.vector.tensor_copy(lgbf[:, j, :], lgT[0:E, :])
                for q in range(4):
                    nc.tensor.transpose(lgt_all[:, ts(4 * j + q, E)],
                                        lgbf[:, j, ts(q, 128)], id8f_sb)

            def ffn_h1(J):
                h1 = ps1.tile([128, TC], F32, tag="hsh")
                for kt in range(8):
                    nc.tensor.matmul(h1, lhsT=sw1t_sb[:, kt, ts(J, 128)],
                                     rhs=xtb_sb[:, kt, :],
                                     start=(kt == 0), stop=(kt == 7))
                nc.scalar.activation(sgall[:, J, :], h1, AF.Silu)

            def ffn_h3(J):
                h3 = ps3.tile([128, TC], F32, tag="h3")
                for kt in range(8):
                    nc.tensor.matmul(h3, lhsT=sw3t_sb[:, kt, ts(J, 128)],
                                     rhs=xtb_sb[:, kt, :],
                                     start=(kt == 0), stop=(kt == 7))
                nc.vector.tensor_mul(hh_sb[:, J, :], sgall[:, J, :], h3)

            # ---- incremental gate DVE pass over 128-token blocks [b0, b0+nb)
            def dve_pass(b0, nb):
                lgv = lgt_all.rearrange("p (b e) -> p b e", e=E)[:, b0:b0 + nb, :]

                def bc8(col):
                    return col.unsqueeze(2).to_broadcast([128, nb, E])

                def bc2(col):
                    return col.unsqueeze(3).to_broadcast([128, nb, G, 2])

                mx = wk.tile([128, nb], F32, tag="mx")
                nc.vector.reduce_max(mx, lgv, axis=X)
                sub = wk.tile([128, nb, E], F32, tag="sub")
                nc.vector.tensor_sub(sub, lgv, bc8(mx))
                ex = wk.tile([128, nb, E], F32, tag="ex")
                nc.scalar.activation(ex, sub, AF.Exp, scale=1.0 / GS)
                sm = wk.tile([128, nb], F32, tag="sm")
                nc.vector.reduce_sum(sm, ex, axis=X)
                rcp = wk.tile([128, nb], F32, tag="rcp")
                nc.vector.reciprocal(rcp, sm)
                scores = wk.tile([128, nb, E], F32, tag="scores")
                nc.vector.tensor_mul(scores, ex, bc8(rcp))
                s = wk.tile([128, nb, E], F32, tag="s")
                nc.vector.tensor_add(s, scores,
                                     bias_sb.unsqueeze(1).to_broadcast([128, nb, E]))
                sv = s.rearrange("p b (g two) -> p b g two", two=2)
                g4 = wk.tile([128, nb, G], F32, tag="g4")
                nc.vector.tensor_add(g4, sv[:, :, :, 0], sv[:, :, :, 1])
                gmax = wk.tile([128, nb], F32, tag="gmax")
                nc.vector.reduce_max(gmax, g4, axis=X)
                ohg1 = wk.tile([128, nb, G], F32, tag="ohg1")
                nc.vector.tensor_tensor(ohg1, g4, bc8(gmax)[:, :, 0:G], op=ALU.is_equal)
                gt = wk.tile([128, nb, G], F32, tag="gt")
                nc.vector.tensor_scalar_mul(gt, ohg1, BIG)
                g2 = wk.tile([128, nb, G], F32, tag="g2")
                nc.vector.tensor_sub(g2, g4, gt)
                gmax2 = wk.tile([128, nb], F32, tag="gmax2")
                nc.vector.reduce_max(gmax2, g2, axis=X)
                ohg2 = wk.tile([128, nb, G], F32, tag="ohg2")
                nc.vector.tensor_tensor(ohg2, g2, bc8(gmax2)[:, :, 0:G],
                                        op=ALU.is_equal)
                keep = wk.tile([128, nb, G], F32, tag="keep")
                nc.vector.tensor_add(keep, ohg1, ohg2)
                mk = wk.tile([128, nb, G], F32, tag="mk")
                nc.vector.tensor_scalar(mk, keep, BIG, BIG,
                                        op0=ALU.mult, op1=ALU.subtract)
                # masked = s*keep + (keep*BIG - BIG)   (exact select)
                m0 = wk.tile([128, nb, G, 2], F32, tag="m0")
                nc.vector.tensor_mul(m0, sv, bc2(keep))
                masked = wk.tile([128, nb, G, 2], F32, tag="masked")
                nc.vector.tensor_add(masked, m0, bc2(mk))
                maskedv = masked.rearrange("p b g two -> p b (g two)")
                m1 = wk.tile([128, nb], F32, tag="m1")
                nc.vector.reduce_max(m1, maskedv, axis=X)
                o1 = oh1v[:, b0:b0 + nb, :]
                nc.vector.tensor_tensor(o1, maskedv, bc8(m1), op=ALU.is_equal)
                t2 = wk.tile([128, nb, E], F32, tag="t2")
                nc.vector.tensor_scalar_mul(t2, o1, BIG)
                masked2 = wk.tile([128, nb, E], F32, tag="masked2")
                nc.vector.tensor_sub(masked2, maskedv, t2)
                m2 = wk.tile([128, nb], F32, tag="m2")
                nc.vector.reduce_max(m2, masked2, axis=X)
                o2 = oh2v[:, b0:b0 + nb, :]
                nc.vector.tensor_tensor(o2, masked2, bc8(m2), op=ALU.is_equal)
                tw1 = wk.tile([128, nb, E], F32, tag="tw1")
                nc.vector.tensor_mul(tw1, o1, scores)
                nc.vector.reduce_sum(wt1[:, b0:b0 + nb], tw1, axis=X)
                tw2 = wk.tile([128, nb, E], F32, tag="tw2")
                nc.vector.tensor_mul(tw2, o2, scores)
                nc.vector.reduce_sum(wt2[:, b0:b0 + nb], tw2, axis=X)

            # ===== statically interleaved PE stream, ordered by DMA arrival
            sched = [('g', 0), ('1', 0), ('1', 1), ('1', 2), ('g', 1), ('1', 3),
                     ('g', 2), ('1', 4), ('g', 3), ('1', 5), ('3', 0), ('1', 6),
                     ('3', 1), ('1', 7), ('3', 2), ('g', 4), ('3', 3), ('g', 5),
                     ('3', 4), ('g', 6), ('3', 5), ('g', 7), ('3', 6), ('3', 7)]
            gates_done = 0
            for kind, idx in sched:
                if kind == 'g':
                    gate_chunk(idx)
                    gates_done += 1
                    if gates_done % 2 == 0:
                        dve_pass(8 * (gates_done // 2 - 1), 8)
                elif kind == '1':
                    ffn_h1(idx)
                else:
                    ffn_h3(idx)

            # ===== global counts -> -offsets, all local =====
            ohs = wk.tile([128, NB, E], F32, tag="ohs")
            nc.vector.tensor_add(ohs, oh1v, oh2v)
            fold = NB
            while fold > 1:
                fold //= 2
                nc.vector.tensor_add(ohs[:, 0:fold, :], ohs[:, 0:fold, :],
                                      ohs[:, fold:2 * fold, :])
            cnt_ps = psg.tile([E, 1], F32, tag="misc")
            nc.tensor.matmul(cnt_ps, lhsT=ohs[:, 0, :], rhs=ones_col,
                             start=True, stop=True)
            cnt_sb = wk.tile([E, 1], F32, tag="cntsb")
            nc.scalar.copy(cnt_sb, cnt_ps)
            # noffs[p, e] = -inclusive_cumsum(cnt)[e], broadcast over partitions
            rhs8 = wk.tile([E, E], F32, tag="rhs8")
            nc.vector.tensor_scalar_mul(rhs8, negL8_sb, cnt_sb)
            noffs_ps = psg.tile([128, E], F32, tag="misc")
            nc.tensor.matmul(noffs_ps, lhsT=ones8_sb, rhs=rhs8, start=True, stop=True)
            noffs = wp.tile([128, E], F32, tag="noffs")
            nc.vector.tensor_copy(noffs, noffs_ps)

            # ===== phi row-sets interleaved with FFN output GEMM tiles =====
            def out_tile(Dt):
                sh = ps1.tile([128, TC], F32, tag="hsh")
                for J in range(8):
                    nc.tensor.matmul(sh, lhsT=sw2t_sb[:, J, ts(Dt, 128)],
                                     rhs=hh_sb[:, J, :],
                                     start=(J == 0), stop=(J == 7))
                o_sb = wk.tile([128, TC], BF16, tag="osbt")
                nc.scalar.copy(o_sb, sh)
                nc.sync.dma_start(out.ap()[ts(Dt, 128), :], o_sb)

            # batched stage-A: one-hot (segment x chosen-expert) masks for all
            # 8 row-sets in a handful of wide DVE ops
            ivJ8 = wk.tile([128, E], F32, tag="ivJ8")   # global row idx per rowset
            nc.vector.tensor_add(ivJ8, ivec_sb.to_broadcast([128, E]), crow_sb)
            Gsum = wk.tile([128, E, E], F32, tag="Gsum")   # [p, rs, e]
            nc.vector.tensor_tensor(Gsum, ivJ8.unsqueeze(2).to_broadcast([128, E, E]),
                                    noffs.unsqueeze(1).to_broadcast([128, E, E]),
                                    op=ALU.add)
            Gm8 = wk.tile([128, E, E], F32, tag="Gm8")
            nc.vector.tensor_scalar(Gm8, Gsum, 0.0, 0.0, op0=ALU.add, op1=ALU.is_ge)
            osb8 = wk.tile([128, E, E], F32, tag="osb8")
            nc.vector.tensor_sub(osb8[:, :, 1:E], Gm8[:, :, 0:E - 1], Gm8[:, :, 1:E])
            nc.vector.tensor_scalar(osb8[:, :, 0:1], Gm8[:, :, 0:1], -1.0, 1.0,
                                    op0=ALU.mult, op1=ALU.add)
            osb8v = osb8.rearrange("p (J k) e -> p J k e", k=2)
            ote8 = []
            for k in range(2):
                ohv = (oh1v if k == 0 else oh2v)
                o8 = gp.tile([128, 4, E * E], BF16, tag=f"ote8k{k}")
                o8v = o8.rearrange("p J (e t) -> p J e t", t=E)
                nc.vector.tensor_tensor(
                    o8v,
                    osb8v[:, :, k, :].unsqueeze(3).to_broadcast([128, 4, E, E]),
                    ohv[:, 0:4, :].unsqueeze(2).to_broadcast([128, 4, E, E]),
                    op=ALU.mult)
                ote8.append(o8)

            # batched transposes of all 8 ote masks -> one PSUM bank -> SBUF
            otTall_ps = psg.tile([E * E, 8 * 128], BF16, tag="misc")
            for rs_i in range(8):
                nc.tensor.transpose(otTall_ps[:, ts(rs_i, 128)],
                                    ote8[rs_i % 2][:, rs_i // 2, :], idbf_sb)
            otT_all = wk.tile([E * E, 8 * 128], BF16, tag="otTall")
            nc.vector.tensor_copy(otT_all, otTall_ps)

            # software-pipelined: H matmul for rowset rs-1 runs while rowset
            # rs's phi is produced on scalar/vector; out tiles fill the PE
            H_ps = psh.tile([E * E, HID], F32, tag="acc")
            phis = []
            for rs_i in range(8):
                k, Jb = rs_i % 2, rs_i // 2
                wtk = (wt1 if k == 0 else wt2)[:, Jb:Jb + 1]
                a_ps = ps3.tile([128, HID], F32, tag="h3")
                nc.tensor.matmul(a_ps, lhsT=otT_all[:, ts(rs_i, 128)], rhs=A_bf,
                                 start=True, stop=True)
                b_ps = ps3.tile([128, HID], F32, tag="h3")
                nc.tensor.matmul(b_ps, lhsT=otT_all[:, ts(rs_i, 128)], rhs=B_bf,
                                 start=True, stop=True)
                if rs_i > 0:
                    pv = rs_i - 1
                    nc.tensor.matmul(H_ps, lhsT=ote8[pv % 2][:, pv // 2, :],
                                     rhs=phis[pv], start=(pv == 0), stop=False)
                sg = wk.tile([128, HID], F32, tag="phia")
                nc.scalar.activation(sg, a_ps, AF.Silu, scale=wtk)
                phi = gp.tile([128, HID], BF16, tag=f"phi{rs_i}")
                nc.vector.scalar_tensor_tensor(phi, b_ps, wtk, sg,
                                               op0=ALU.mult, op1=ALU.mult)
                phis.append(phi)
                out_tile(rs_i)
            nc.tensor.matmul(H_ps, lhsT=ote8[1][:, 3, :], rhs=phis[7],
                             start=False, stop=True)
            H_sb = wk.tile([E * E, HID], F32, tag="Hsb")
            nc.vector.tensor_copy(H_sb, H_ps)
            nc.sync.dma_start(hout.ap(), H_sb)

    nc.compile()
    return nc


_NC = None


def _get_nc():
    global _NC
    if _NC is None:
        _NC = build()
    return _NC


def _pack(a, k):
    """[k*128, f] -> [128, k, f] partition-major contiguous."""
    kk, f = a.shape
    assert kk == k * 128
    return np.ascontiguousarray(a.reshape(k, 128, f).transpose(1, 0, 2))


def make_in_maps(x, w_gate, w1, w2, w3, sw1, sw2, sw3, expert_bias):
    bf = ml_dtypes.bfloat16
    f8 = ml_dtypes.float8_e4m3fn
    xf = np.ascontiguousarray(np.asarray(x, np.float32).reshape(NTOK, D))
    xT = np.ascontiguousarray(xf.T)                       # [D, NTOK]
    wgp = np.zeros((D, 2 * E), np.float32)
    wgp[:, :E] = np.asarray(w_gate, np.float32).T * GS
    wg8_np = _pack(wgp.astype(f8), 8)
    sw1t_np = _pack(np.ascontiguousarray(np.asarray(sw1, np.float32).T).astype(bf), 8)
    sw3t_np = _pack(np.ascontiguousarray(np.asarray(sw3, np.float32).T).astype(bf), 8)
    sw2t_np = _pack(np.ascontiguousarray(np.asarray(sw2, np.float32).T).astype(bf), 8)
    bias_np = np.ascontiguousarray(np.asarray(expert_bias, np.float32).reshape(1, E))
    # host tables: A[8e+t] = x[t] @ w1[e], B likewise with w3   [64, 512] each
    w1_np = np.asarray(w1, np.float32)
    w3_np = np.asarray(w3, np.float32)
    x8 = xf[:E]                                           # [8, D]
    A = np.einsum('td,edh->eth', x8, w1_np).reshape(E * E, HID)
    B = np.einsum('td,edh->eth', x8, w3_np).reshape(E * E, HID)
    tabs_np = np.ascontiguousarray(
        np.concatenate([A, B], axis=1).astype(bf))        # [64, 1024]
    # per-core x: own 512-token block first, then the other blocks in order
    xt8_pk = _pack(xT.astype(f8), 8)                      # [128, 8, NTOK] fp8
    xt_pk = _pack(xT.astype(bf), 8)                       # [128, 8, NTOK] bf16
    in_maps = []
    for c in range(C):
        order = [c] + [j for j in range(8) if j != c]
        x8a_np = np.ascontiguousarray(
            xt8_pk.reshape(128, 8, 8, TC)[:, :, order, :].transpose(0, 2, 1, 3)
            .reshape(128, 8, 8 * TC))
        xtb_np = np.ascontiguousarray(xt_pk[:, :, c * TC:(c + 1) * TC])
        in_maps.append({
            "x8a": x8a_np,
            "xtb": xtb_np,
            "wg8": wg8_np,
            "sw1t": sw1t_np,
            "sw3t": sw3t_np,
            "sw2t": sw2t_np,
            "tabs": tabs_np,
            "biasd": bias_np,
            "ivec": (1024.0 * c + 2.0 * np.arange(128, dtype=np.float32)).reshape(128, 1),
        })
    return in_maps


def combine_outputs(results, w2):
    full = np.empty((NTOK, D), np.float32)
    Hsum = np.zeros((E * E, HID), np.float32)
    for c in range(C):
        full[c * TC:(c + 1) * TC] = results[c]["out"].T.astype(np.float32)
        Hsum += results[c]["hout"]
    # delta[t] = sum_e H[8e+t] @ w2[e]   (t-major rows @ stacked w2)
    Ht = Hsum.reshape(E, E, HID).transpose(1, 0, 2).reshape(E, E * HID)
    delta = Ht @ np.asarray(w2, np.float32).reshape(E * HID, D)
    full[:E] += delta
    return full.reshape(2, 2048, D)


def kernel(x, w_gate, w1, w2, w3, sw1, sw2, sw3, expert_bias, **_unused):
    nc = _get_nc()
    in_maps = make_in_maps(x, w_gate, w1, w2, w3, sw1, sw2, sw3, expert_bias)
    res = bass_utils.run_bass_kernel_spmd(nc, in_maps, core_ids=list(range(C)))
    return combine_outputs(res.results, w2)


# revision 5
# speedup vs baseline: 1.2095x; 1.0034x over previous
"""Trainium2 Bass kernel for grouped-top-k MoE with shared expert (8 NeuronCores, SPMD).

Zero-collective design
----------------------
The reference's "dispatch" gathers rows of x by *expert id* (values 0..7), so the
routed path only ever reads x[0:8] and scatter-adds into output rows 0..7.  Writing
routed_out row i as g(w_i * x[t_i]; e_i) with t_i = chosen expert of assignment i and
e_i = ragged-segment expert of global row i, the whole routed computation factors
through a 64-row table:
    a[t,e] = x[t] @ w1[e],  b[t,e] = x[t] @ w3[e]            (host precompute)
    H[t,e] = sum_{i: t_i=t, e_i=e} silu(w_i*a[t,e]) * (w_i*b[t,e])
    delta[t] = sum_e H[t,e] @ w2[e];   out[t] += delta[t]  (t < 8, host combine)

No collectives at all (v1's three collectives cost 75us + a 42us rendezvous
barrier for <160KB of payload):
  - every core computes the GATE for all 4096 tokens, so global expert
    counts/offsets are available locally.  The gate matmuls run in fp8
    (DoubleRow, 2x) on a 16x-prescaled w_gate; the softmax Exp rescales.
    Routing differs from the f32 reference on ~160/4096 near-tie tokens, which
    only perturbs the 8 delta rows (measured ~0.009 total rel err vs 2e-2 gate).
  - the 64x512 a/b tables are computed on host during input packing and DMA'd.
  - each core emits its partial H [64,512] f32; the host sums the 8 partials
    and applies the tiny w2 GEMM in f32 during unshard (the v1 kernel already
    host-summed partial deltas).
  - data-parallel over tokens for the shared-expert FFN (512 tokens/core);
    per-core x is packed with the OWN 512-token block first so one NEFF serves
    all cores.

Scheduling: one statically interleaved PE stream ordered by DMA arrival -
h1 tiles (sw1t) first with gate chunks riding the fp8 x blocks, h3 tiles after
sw3t, alternating tags so PSUM rotations never stall; the gate DVE chain runs
in 4 incremental passes so counts are ready right after the last gate chunk;
phi/H then out-GEMMs form the tail.
"""

import sys

if "/opt/trn_rl_repo" not in sys.path:
    sys.path.insert(0, "/opt/trn_rl_repo")

import numpy as np
import ml_dtypes

import concourse.bass as bass
import concourse.mybir as mybir
import concourse.tile as tile
from concourse import bacc
from concourse import bass_utils

F32 = mybir.dt.float32
BF16 = mybir.dt.bfloat16
FP8 = mybir.dt.float8e4
DR = mybir.MatmulPerfMode.DoubleRow
GS = 16.0   # gate fp8 weight pre-scale (softmax Exp divides it back out)
AF = mybir.ActivationFunctionType
ALU = mybir.AluOpType
X = mybir.AxisListType.X

E = 8          # experts (== table token count == cores)
G = 4          # expert groups
D = 1024       # model dim
HID = 512      # expert hidden
SH = 1024      # shared-expert hidden
C = 8          # cores
TC = 512       # tokens per core
NTOK = 4096
NB = 32        # 128-token blocks globally
BIG = 1.0e30


def ts(i, s):
    return slice(i * s, (i + 1) * s)


def build():
    nc = bacc.Bacc("TRN2", target_bir_lowering=False, debug=False, num_devices=C)

    # ---- I/O: packed partition-major; contraction dim = k*128+p
    wg8 = nc.dram_tensor("wg8", [128, 8, 2 * E], FP8, kind="ExternalInput")
    biasd = nc.dram_tensor("biasd", [1, E], F32, kind="ExternalInput")
    ivec = nc.dram_tensor("ivec", [128, 1], F32, kind="ExternalInput")
    # all 4096 tokens in fp8 (gate only), dim-major; block 0 = own shard
    x8a = nc.dram_tensor("x8a", [128, 8, 8 * TC], FP8, kind="ExternalInput")  # block-major
    xtb = nc.dram_tensor("xtb", [128, 8, TC], BF16, kind="ExternalInput")
    sw1t = nc.dram_tensor("sw1t", [128, 8, SH], BF16, kind="ExternalInput")
    sw3t = nc.dram_tensor("sw3t", [128, 8, SH], BF16, kind="ExternalInput")
    sw2t = nc.dram_tensor("sw2t", [128, 8, D], BF16, kind="ExternalInput")
    tabs_d = nc.dram_tensor("tabs", [E * E, 2 * HID], BF16, kind="ExternalInput")
    out = nc.dram_tensor("out", [D, TC], BF16, kind="ExternalOutput")  # shared^T shard
    hout = nc.dram_tensor("hout", [E * E, HID], F32, kind="ExternalOutput")  # partial H

    # ---- compile-time constants (embedded in NEFF)
    idbf_d = nc.inline_tensor(np.eye(128, dtype=ml_dtypes.bfloat16), name="idbf")
    id8f_d = nc.inline_tensor(np.eye(E, dtype=np.float32), name="id8f")
    # negL8[k, e] = -1 if k <= e else 0;  noffs[e] = sum_k negL8[k,e]*cnt[k]
    negL8_d = nc.inline_tensor(
        np.ascontiguousarray(-np.tril(np.ones((E, E), np.float32)).T), name="negL8")
    ones8_d = nc.inline_tensor(np.ones((E, 128), np.float32), name="ones8x128")
    crow_d = nc.inline_tensor(
        np.array([[256 * Jb + k for Jb in range(4) for k in range(2)]],
                 np.float32).reshape(1, E), name="crow")

    with tile.TileContext(nc) as tc:
        with (
            tc.tile_pool(name="wp", bufs=1) as wp,       # persistent SBUF
            tc.tile_pool(name="gp", bufs=1) as gp,       # gate outputs (persist to phi)
            tc.tile_pool(name="wk", bufs=2) as wk,       # transient SBUF
            tc.tile_pool(name="psg", bufs=1, space="PSUM") as psg,   # transposes/misc
            tc.tile_pool(name="psl", bufs=1, space="PSUM") as psl,   # gate logit chunks
            tc.tile_pool(name="psh", bufs=1, space="PSUM") as psh,   # H accumulator
            tc.tile_pool(name="ps1", bufs=3, space="PSUM") as ps1,   # h1 + out tiles
            tc.tile_pool(name="ps3", bufs=2, space="PSUM") as ps3,   # h3 + phi a/b
        ):
            # ===== loads, ordered by first use (kt-halves so tiles start early)
            wg8_sb = wp.tile([128, 8, 2 * E], FP8, tag="wg8")
            nc.sync.dma_start(wg8_sb, wg8.ap())
            x8a_sb = wp.tile([128, 8, 8 * TC], FP8, tag="x8a")  # [p, blk, kt*tok]
            nc.sync.dma_start(x8a_sb[:, 0, :], x8a.ap()[:, 0, :])
            id8f_sb = wp.tile([E, E], F32, tag="id8f")
            nc.sync.dma_start(id8f_sb, id8f_d.ap())
            sw1t_sb = wp.tile([128, 8, SH], BF16, tag="sw1t")
            xtb_sb = wp.tile([128, 8, TC], BF16, tag="xtb")
            for q in range(4):
                nc.sync.dma_start(sw1t_sb[:, 2 * q:2 * q + 2, :],
                                  sw1t.ap()[:, 2 * q:2 * q + 2, :])
                nc.sync.dma_start(xtb_sb[:, 2 * q:2 * q + 2, :],
                                  xtb.ap()[:, 2 * q:2 * q + 2, :])
            bias_sb = wp.tile([128, E], F32, tag="bias")
            nc.sync.dma_start(bias_sb, biasd.ap().to_broadcast([128, E]))
            for j in range(1, 4):
                nc.sync.dma_start(x8a_sb[:, j, :], x8a.ap()[:, j, :])
            sw3t_sb = wp.tile([128, 8, SH], BF16, tag="sw3t")
            nc.sync.dma_start(sw3t_sb[:, 0:4, :], sw3t.ap()[:, 0:4, :])
            for j in range(4, 6):
                nc.sync.dma_start(x8a_sb[:, j, :], x8a.ap()[:, j, :])
            nc.sync.dma_start(sw3t_sb[:, 4:8, :], sw3t.ap()[:, 4:8, :])
            for j in range(6, 8):
                nc.sync.dma_start(x8a_sb[:, j, :], x8a.ap()[:, j, :])
            tabs = wp.tile([E * E, 2 * HID], BF16, tag="tabs")
            nc.sync.dma_start(tabs, tabs_d.ap())
            sw2t_sb = wp.tile([128, 8, D], BF16, tag="sw2t")
            nc.sync.dma_start(sw2t_sb, sw2t.ap())
            # small late-use constants
            ivec_sb = wp.tile([128, 1], F32, tag="ivec")
            nc.sync.dma_start(ivec_sb, ivec.ap())
            idbf_sb = wp.tile([128, 128], BF16, tag="idbf")
            nc.sync.dma_start(idbf_sb, idbf_d.ap())
            negL8_sb = wp.tile([E, E], F32, tag="negL8")
            nc.sync.dma_start(negL8_sb, negL8_d.ap())
            ones8_sb = wp.tile([E, 128], F32, tag="ones8")
            nc.sync.dma_start(ones8_sb, ones8_d.ap())
            crow_sb = wp.tile([128, E], F32, tag="crow")
            nc.sync.dma_start(crow_sb, crow_d.ap().to_broadcast([128, E]))
            ones_col = wp.tile([128, 1], F32, tag="ones_col")
            nc.vector.memset(ones_col, 1.0)

            A_bf = tabs[:, 0:HID]
            B_bf = tabs[:, HID:2 * HID]

            # ===== persistent gate-phase tiles =====
            lgbf = wp.tile([E, 8, TC], F32, tag="lgbf")
            lgt_all = psg.tile([128, NB * E], F32, tag="misc")
            hh_sb = wp.tile([128, 8, TC], BF16, tag="hh")
            sgall = wp.tile([128, 8, TC], F32, tag="sgall")
            oh1 = gp.tile([128, NB * E], F32, tag="oh1all")
            oh1v = oh1.rearrange("p (b e) -> p b e", e=E)
            oh2 = gp.tile([128, NB * E], F32, tag="oh2all")
            oh2v = oh2.rearrange("p (b e) -> p b e", e=E)
            wt1 = gp.tile([128, NB], F32, tag="wt1all")
            wt2 = gp.tile([128, NB], F32, tag="wt2all")

            def gate_chunk(j):
                xj = x8a_sb[:, j, :].rearrange("p (k t) -> p k t", t=TC)
                lgT = psl.tile([2 * E, TC], F32, tag="lgT")
                for q in range(4):
                    nc.tensor.matmul(lgT, lhsT=wg8_sb[:, 2 * q:2 * q + 2, :],
                                     rhs=xj[:, 2 * q:2 * q + 2, :],
                                     start=(q == 0), stop=(q == 3), perf_mode=DR)
                nc.vector.tensor_copy(lgbf[:, j, :], lgT[0:E, :])
                for q in range(4):
                    nc.tensor.transpose(lgt_all[:, ts(4 * j + q, E)],
                                        lgbf[:, j, ts(q, 128)], id8f_sb)

            def ffn_h1(J):
                h1 = ps1.tile([128, TC], F32, tag="hsh")
                for kt in range(8):
                    nc.tensor.matmul(h1, lhsT=sw1t_sb[:, kt, ts(J, 128)],
                                     rhs=xtb_sb[:, kt, :],
                                     start=(kt == 0), stop=(kt == 7))
                nc.scalar.activation(sgall[:, J, :], h1, AF.Silu)

            def ffn_h3(J):
                h3 = ps3.tile([128, TC], F32, tag="h3")
                for kt in range(8):
                    nc.tensor.matmul(h3, lhsT=sw3t_sb[:, kt, ts(J, 128)],
                                     rhs=xtb_sb[:, kt, :],
                                     start=(kt == 0), stop=(kt == 7))
                nc.vector.tensor_mul(hh_sb[:, J, :], sgall[:, J, :], h3)

            # ---- incremental gate DVE pass over 128-token blocks [b0, b0+nb)
            def dve_pass(b0, nb):
                lgv = lgt_all.rearrange("p (b e) -> p b e", e=E)[:, b0:b0 + nb, :]

                def bc8(col):
                    return col.unsqueeze(2).to_broadcast([128, nb, E])

                def bc2(col):
                    return col.unsqueeze(3).to_broadcast([128, nb, G, 2])

                mx = wk.tile([128, nb], F32, tag="mx")
                nc.vector.reduce_max(mx, lgv, axis=X)
                sub = wk.tile([128, nb, E], F32, tag="sub")
                nc.vector.tensor_sub(sub, lgv, bc8(mx))
                ex = wk.tile([128, nb, E], F32, tag="ex")
                nc.scalar.activation(ex, sub, AF.Exp, scale=1.0 / GS)
                sm = wk.tile([128, nb], F32, tag="sm")
                nc.vector.reduce_sum(sm, ex, axis=X)
                rcp = wk.tile([128, nb], F32, tag="rcp")
                nc.vector.reciprocal(rcp, sm)
                scores = wk.tile([128, nb, E], F32, tag="scores")
                nc.vector.tensor_mul(scores, ex, bc8(rcp))
                s = wk.tile([128, nb, E], F32, tag="s")
                nc.vector.tensor_add(s, scores,
                                     bias_sb.unsqueeze(1).to_broadcast([128, nb, E]))
                sv = s.rearrange("p b (g two) -> p b g two", two=2)
                g4 = wk.tile([128, nb, G], F32, tag="g4")
                nc.vector.tensor_add(g4, sv[:, :, :, 0], sv[:, :, :, 1])
                gmax = wk.tile([128, nb], F32, tag="gmax")
                nc.vector.reduce_max(gmax, g4, axis=X)
                ohg1 = wk.tile([128, nb, G], F32, tag="ohg1")
                nc.vector.tensor_tensor(ohg1, g4, bc8(gmax)[:, :, 0:G], op=ALU.is_equal)
                gt = wk.tile([128, nb, G], F32, tag="gt")
                nc.vector.tensor_scalar_mul(gt, ohg1, BIG)
                g2 = wk.tile([128, nb, G], F32, tag="g2")
                nc.vector.tensor_sub(g2, g4, gt)
                gmax2 = wk.tile([128, nb], F32, tag="gmax2")
                nc.vector.reduce_max(gmax2, g2, axis=X)
                ohg2 = wk.tile([128, nb, G], F32, tag="ohg2")
                nc.vector.tensor_tensor(ohg2, g2, bc8(gmax2)[:, :, 0:G],
                                        op=ALU.is_equal)
                keep = wk.tile([128, nb, G], F32, tag="keep")
                nc.vector.tensor_add(keep, ohg1, ohg2)
                mk = wk.tile([128, nb, G], F32, tag="mk")
                nc.vector.tensor_scalar(mk, keep, BIG, BIG,
                                        op0=ALU.mult, op1=ALU.subtract)
                # masked = s*keep + (keep*BIG - BIG)   (exact select)
                m0 = wk.tile([128, nb, G, 2], F32, tag="m0")
                nc.vector.tensor_mul(m0, sv, bc2(keep))
                masked = wk.tile([128, nb, G, 2], F32, tag="masked")
                nc.vector.tensor_add(masked, m0, bc2(mk))
                maskedv = masked.rearrange("p b g two -> p b (g two)")
                m1 = wk.tile([128, nb], F32, tag="m1")
                nc.vector.reduce_max(m1, maskedv, axis=X)
                o1 = oh1v[:, b0:b0 + nb, :]
                nc.vector.tensor_tensor(o1, maskedv, bc8(m1), op=ALU.is_equal)
                t2 = wk.tile([128, nb, E], F32, tag="t2")
                nc.vector.tensor_scalar_mul(t2, o1, BIG)
                masked2 = wk.tile([128, nb, E], F32, tag="masked2")
                nc.vector.tensor_sub(masked2, maskedv, t2)
                m2 = wk.tile([128, nb], F32, tag="m2")
                nc.vector.reduce_max(m2, masked2, axis=X)
                o2 = oh2v[:, b0:b0 + nb, :]
                nc.vector.tensor_tensor(o2, masked2, bc8(m2), op=ALU.is_equal)
                tw1 = wk.tile([128, nb, E], F32, tag="tw1")
                nc.vector.tensor_mul(tw1, o1, scores)
                nc.vector.reduce_sum(wt1[:, b0:b0 + nb], tw1, axis=X)
                tw2 = wk.tile([128, nb, E], F32, tag="tw2")
                nc.vector.tensor_mul(tw2, o2, scores)
                nc.vector.reduce_sum(wt2[:, b0:b0 + nb], tw2, axis=X)

            # ===== statically interleaved PE stream, ordered by DMA arrival
            sched = [('g', 0), ('1', 0), ('1', 1), ('1', 2), ('g', 1), ('1', 3),
                     ('g', 2), ('1', 4), ('g', 3), ('1', 5), ('3', 0), ('1', 6),
                     ('3', 1), ('1', 7), ('3', 2), ('g', 4), ('3', 3), ('g', 5),
                     ('3', 4), ('g', 6), ('3', 5), ('g', 7), ('3', 6), ('3', 7)]
            gates_done = 0
            for kind, idx in sched:
                if kind == 'g':
                    gate_chunk(idx)
                    gates_done += 1
                    if gates_done % 2 == 0:
                        dve_pass(8 * (gates_done // 2 - 1), 8)
                elif kind == '1':
                    ffn_h1(idx)
                else:
                    ffn_h3(idx)

            # ===== global counts -> -offsets, all local =====
            # ohs written expert-major so one X-reduce sums the 32 blocks
            ohsT = wk.tile([128, E, NB], F32, tag="ohsT")
            nc.vector.tensor_add(ohsT.rearrange("p e b -> p b e"), oh1v, oh2v)
            ohsum = wk.tile([128, E], F32, tag="ohsum")
            nc.vector.reduce_sum(ohsum, ohsT, axis=X)
            cnt_ps = psg.tile([E, 1], F32, tag="misc")
            nc.tensor.matmul(cnt_ps, lhsT=ohsum, rhs=ones_col,
                             start=True, stop=True)
            cnt_sb = wk.tile([E, 1], F32, tag="cntsb")
            nc.vector.tensor_copy(cnt_sb, cnt_ps)
            # noffs[p, e] = -inclusive_cumsum(cnt)[e], broadcast over partitions
            rhs8 = wk.tile([E, E], F32, tag="rhs8")
            nc.vector.tensor_scalar_mul(rhs8, negL8_sb, cnt_sb)
            noffs_ps = psg.tile([128, E], F32, tag="misc")
            nc.tensor.matmul(noffs_ps, lhsT=ones8_sb, rhs=rhs8, start=True, stop=True)
            noffs = noffs_ps

            # ===== phi row-sets interleaved with FFN output GEMM tiles =====
            def out_tile(Dt):
                sh = ps1.tile([128, TC], F32, tag="hsh")
                for J in range(8):
                    nc.tensor.matmul(sh, lhsT=sw2t_sb[:, J, ts(Dt, 128)],
                                     rhs=hh_sb[:, J, :],
                                     start=(J == 0), stop=(J == 7))
                o_sb = wk.tile([128, TC], BF16, tag="osbt")
                nc.scalar.copy(o_sb, sh)
                nc.sync.dma_start(out.ap()[ts(Dt, 128), :], o_sb)

            # batched stage-A: one-hot (segment x chosen-expert) masks for all
            # 8 row-sets in a handful of wide DVE ops
            ivJ8 = wk.tile([128, E], F32, tag="ivJ8")   # global row idx per rowset
            nc.vector.tensor_add(ivJ8, ivec_sb.to_broadcast([128, E]), crow_sb)
            Gsum = wk.tile([128, E, E], F32, tag="Gsum")   # [p, rs, e]
            nc.vector.tensor_tensor(Gsum, ivJ8.unsqueeze(2).to_broadcast([128, E, E]),
                                    noffs.unsqueeze(1).to_broadcast([128, E, E]),
                                    op=ALU.add)
            Gm8 = wk.tile([128, E, E], F32, tag="Gm8")
            nc.vector.tensor_scalar(Gm8, Gsum, 0.0, 0.0, op0=ALU.add, op1=ALU.is_ge)
            osb8 = wk.tile([128, E, E], F32, tag="osb8")
            nc.vector.tensor_sub(osb8[:, :, 1:E], Gm8[:, :, 0:E - 1], Gm8[:, :, 1:E])
            nc.vector.tensor_scalar(osb8[:, :, 0:1], Gm8[:, :, 0:1], -1.0, 1.0,
                                    op0=ALU.mult, op1=ALU.add)
            osb8v = osb8.rearrange("p (J k) e -> p J k e", k=2)
            ote8 = []
            for k in range(2):
                ohv = (oh1v if k == 0 else oh2v)
                o8 = gp.tile([128, 4, E * E], BF16, tag=f"ote8k{k}")
                o8v = o8.rearrange("p J (e t) -> p J e t", t=E)
                nc.vector.tensor_tensor(
                    o8v,
                    osb8v[:, :, k, :].unsqueeze(3).to_broadcast([128, 4, E, E]),
                    ohv[:, 0:4, :].unsqueeze(2).to_broadcast([128, 4, E, E]),
                    op=ALU.mult)
                ote8.append(o8)

            # batched transposes of all 8 ote masks -> one PSUM bank -> SBUF
            otTall_ps = psg.tile([E * E, 8 * 128], BF16, tag="misc")
            for rs_i in range(8):
                nc.tensor.transpose(otTall_ps[:, ts(rs_i, 128)],
                                    ote8[rs_i % 2][:, rs_i // 2, :], idbf_sb)
            otT_all = wk.tile([E * E, 8 * 128], BF16, tag="otTall")
            nc.vector.tensor_copy(otT_all, otTall_ps)

            # software-pipelined: H matmul for rowset rs-1 runs while rowset
            # rs's phi is produced on scalar/vector; out tiles fill the PE
            H_ps = psh.tile([E * E, HID], F32, tag="acc")
            phis = []
            for rs_i in range(8):
                k, Jb = rs_i % 2, rs_i // 2
                wtk = (wt1 if k == 0 else wt2)[:, Jb:Jb + 1]
                a_ps = ps3.tile([128, HID], F32, tag="h3")
                nc.tensor.matmul(a_ps, lhsT=otT_all[:, ts(rs_i, 128)], rhs=A_bf,
                                 start=True, stop=True)
                b_ps = ps3.tile([128, HID], F32, tag="h3")
                nc.tensor.matmul(b_ps, lhsT=otT_all[:, ts(rs_i, 128)], rhs=B_bf,
                                 start=True, stop=True)
                if rs_i > 0:
                    pv = rs_i - 1
                    nc.tensor.matmul(H_ps, lhsT=ote8[pv % 2][:, pv // 2, :],
                                     rhs=phis[pv], start=(pv == 0), stop=False)
                sg = wk.tile([128, HID], F32, tag="phia")
                nc.scalar.activation(sg, a_ps, AF.Silu, scale=wtk)
                phi = gp.tile([128, HID], BF16, tag=f"phi{rs_i}")
                nc.vector.scalar_tensor_tensor(phi, b_ps, wtk, sg,
                                               op0=ALU.mult, op1=ALU.mult)
                phis.append(phi)
                out_tile(rs_i)
            nc.tensor.matmul(H_ps, lhsT=ote8[1][:, 3, :], rhs=phis[7],
                             start=False, stop=True)
            H_sb = wk.tile([E * E, HID], F32, tag="Hsb")
            nc.vector.tensor_copy(H_sb, H_ps)
            nc.sync.dma_start(hout.ap(), H_sb)

    nc.compile()
    return nc


_NC = None


def _get_nc():
    global _NC
    if _NC is None:
        _NC = build()
    return _NC


def _pack(a, k):
    """[k*128, f] -> [128, k, f] partition-major contiguous."""
    kk, f = a.shape
    assert kk == k * 128
    return np.ascontiguousarray(a.reshape(k, 128, f).transpose(1, 0, 2))


def make_in_maps(x, w_gate, w1, w2, w3, sw1, sw2, sw3, expert_bias):
    bf = ml_dtypes.bfloat16
    f8 = ml_dtypes.float8_e4m3fn
    xf = np.ascontiguousarray(np.asarray(x, np.float32).reshape(NTOK, D))
    xT = np.ascontiguousarray(xf.T)                       # [D, NTOK]
    wgp = np.zeros((D, 2 * E), np.float32)
    wgp[:, :E] = np.asarray(w_gate, np.float32).T * GS
    wg8_np = _pack(wgp.astype(f8), 8)
    sw1t_np = _pack(np.ascontiguousarray(np.asarray(sw1, np.float32).T).astype(bf), 8)
    sw3t_np = _pack(np.ascontiguousarray(np.asarray(sw3, np.float32).T).astype(bf), 8)
    sw2t_np = _pack(np.ascontiguousarray(np.asarray(sw2, np.float32).T).astype(bf), 8)
    bias_np = np.ascontiguousarray(np.asarray(expert_bias, np.float32).reshape(1, E))
    # host tables: A[8e+t] = x[t] @ w1[e], B likewise with w3   [64, 512] each
    w1_np = np.asarray(w1, np.float32)
    w3_np = np.asarray(w3, np.float32)
    x8 = xf[:E]                                           # [8, D]
    A = np.einsum('td,edh->eth', x8, w1_np).reshape(E * E, HID)
    B = np.einsum('td,edh->eth', x8, w3_np).reshape(E * E, HID)
    tabs_np = np.ascontiguousarray(
        np.concatenate([A, B], axis=1).astype(bf))        # [64, 1024]
    # per-core x: own 512-token block first, then the other blocks in order
    xt8_pk = _pack(xT.astype(f8), 8)                      # [128, 8, NTOK] fp8
    xt_pk = _pack(xT.astype(bf), 8)                       # [128, 8, NTOK] bf16
    in_maps = []
    for c in range(C):
        order = [c] + [j for j in range(8) if j != c]
        x8a_np = np.ascontiguousarray(
            xt8_pk.reshape(128, 8, 8, TC)[:, :, order, :].transpose(0, 2, 1, 3)
            .reshape(128, 8, 8 * TC))
        xtb_np = np.ascontiguousarray(xt_pk[:, :, c * TC:(c + 1) * TC])
        in_maps.append({
            "x8a": x8a_np,
            "xtb": xtb_np,
            "wg8": wg8_np,
            "sw1t": sw1t_np,
            "sw3t": sw3t_np,
            "sw2t": sw2t_np,
            "tabs": tabs_np,
            "biasd": bias_np,
            "ivec": (1024.0 * c + 2.0 * np.arange(128, dtype=np.float32)).reshape(128, 1),
        })
    return in_maps


def combine_outputs(results, w2):
    full = np.empty((NTOK, D), np.float32)
    Hsum = np.zeros((E * E, HID), np.float32)
    for c in range(C):
        full[c * TC:(c + 1) * TC] = results[c]["out"].T.astype(np.float32)
        Hsum += results[c]["hout"]
    # delta[t] = sum_e H[8e+t] @ w2[e]   (t-major rows @ stacked w2)
    Ht = Hsum.reshape(E, E, HID).transpose(1, 0, 2).reshape(E, E * HID)
    delta = Ht @ np.asarray(w2, np.float32).reshape(E * HID, D)
    full[:E] += delta
    return full.reshape(2, 2048, D)


def kernel(x, w_gate, w1, w2, w3, sw1, sw2, sw3, expert_bias, **_unused):
    nc = _get_nc()
    in_maps = make_in_maps(x, w_gate, w1, w2, w3, sw1, sw2, sw3, expert_bias)
    res = bass_utils.run_bass_kernel_spmd(nc, in_maps, core_ids=list(range(C)))
    return combine_outputs(res.results, w2)
